# revision 54
# baseline (speedup 1.0000x reference)
"""MADE autoregressive sampler on 8 TRN2 NeuronCores — incremental frontier.

Strategy (vs. the full-recompute baseline):
- Data-parallel over batch: B=4096 -> 512 rows per core; weights replicated.
- Degree-sort hidden units. In MADE, a hidden unit's activation is FINAL once
  x columns 0..deg are set, so per AR step only the 1-2 "frontier" blocks
  (those containing degree idx-1) need recomputation. Everything else is
  computed once and cached:
    * z1 (layer-1 preact) kept in PSUM, updated by a rank-1 matmul per step.
    * S2/S3 = frozen off-diagonal partial sums per frontier block, cached in
      SBUF and restored into PSUM each step (then diag matmul accumulates).
    * theta (output-layer contributions of finalized blocks) accumulates in
      one PSUM bank, in batch-major chunk layout so the per-step tail ops are
      [128, 4] instead of [1, 512].
- fp16 operands everywhere (fp32 PSUM accumulation).
- Elementwise load spread across Scalar(Act)/Vector(DVE)/Pool(gpsimd).
"""

import os
import sys
import math
import hashlib
import numpy as np

for _p in ("/opt/trn_rl_repo", "/opt/pypackages"):
    if _p not in sys.path:
        sys.path.insert(0, _p)

import concourse.bass as bass
import concourse.tile as tile
from concourse import bacc
from concourse import mybir
from concourse.bass_utils import run_bass_kernel_spmd

D, H, L, B = 32, 1024, 2, 4096
NCORES = 8
BC = B // NCORES          # 512 batch rows per core
P = 128
NB = H // P               # 8 hidden blocks
NJ = BC // P              # 4 batch chunks of 128
F32 = mybir.dt.float32
F16 = mybir.dt.float16

DTYPE = os.environ.get("MADE_DTYPE", "fp16")
MMDT = {"fp16": mybir.dt.float16, "bf16": mybir.dt.bfloat16,
        "f32r": mybir.dt.float32r}[DTYPE]
NPDT = {"fp16": np.float16, "bf16": np.float32, "f32r": np.float32}[DTYPE]
STOP = int(os.environ.get("MADE_STOP", "32"))

AluOp = mybir.AluOpType
ActFn = mybir.ActivationFunctionType


def _schedule():
    """Static per-step schedule from the degree structure."""
    d_hid = np.arange(H) % (D - 1)
    perm = np.argsort(d_hid, kind="stable")
    ds = d_hid[perm]
    g_lo = [int(ds[P * b]) for b in range(NB)]
    g_hi = [int(ds[P * b + P - 1]) for b in range(NB)]
    entry = [g_lo[b] + 1 for b in range(NB)]
    final = [g_hi[b] + 1 for b in range(NB)]
    return perm, ds, g_lo, g_hi, entry, final


def _host_prep(W0, b0, Wh, bh, Wout, bout):
    d_in = np.arange(D)
    d_hid = np.arange(H) % (D - 1)
    d_out = np.arange(D) - 1
    m0 = (d_hid[:, None] >= d_in[None, :]).astype(np.float32)
    mh = (d_hid[:, None] >= d_hid[None, :]).astype(np.float32)
    mo = (d_out[:, None] >= d_hid[None, :]).astype(np.float32)
    mo = np.concatenate([mo, mo], axis=0)

    perm, ds, g_lo, g_hi, entry, final = _schedule()

    W0p = (m0 * W0)[perm]                     # [H, D]
    Wh0p = (mh * Wh[0])[perm][:, perm]        # [H, H] (out, in)
    Wh1p = (mh * Wh[1])[perm][:, perm]
    Wop = (mo * Wout)[:, perm]                # [2D, H]
    b0p = b0[perm]
    bh0p = bh[0][perm]
    bh1p = bh[1][perm]

    # lhsT layouts
    W0T = np.ascontiguousarray(W0p.T).astype(NPDT)          # [32, H]
    Wh0T = np.ascontiguousarray(
        Wh0p.T.reshape(NB, P, H).transpose(1, 0, 2)).astype(NPDT)  # [128, NB, H]
    Wh1T = np.ascontiguousarray(
        Wh1p.T.reshape(NB, P, H).transpose(1, 0, 2)).astype(NPDT)
    # interleaved output weights: col 2i = mu_i, col 2i+1 = ls_i
    WoI = np.empty((H, 2 * D), dtype=np.float32)
    WoI[:, 0::2] = Wop[:D, :].T
    WoI[:, 1::2] = Wop[D:, :].T
    WoIT = np.ascontiguousarray(
        WoI.reshape(NB, P, 2 * D).transpose(1, 0, 2)).astype(NPDT)  # [128, NB, 64]

    b0L = np.ascontiguousarray(b0p.reshape(NB, P).T).astype(np.float32)
    bh0L = np.ascontiguousarray(bh0p.reshape(NB, P).T).astype(np.float32)
    bh1L = np.ascontiguousarray(bh1p.reshape(NB, P).T).astype(np.float32)

    nzh0 = np.zeros((NB, NB), dtype=bool)
    nzh1 = np.zeros((NB, NB), dtype=bool)
    for r in range(NB):
        for c in range(NB):
            nzh0[r, c] = bool(np.any(Wh0p[r * P:(r + 1) * P, c * P:(c + 1) * P]))
            nzh1[r, c] = bool(np.any(Wh1p[r * P:(r + 1) * P, c * P:(c + 1) * P]))

    return dict(W0T=W0T, Wh0T=Wh0T, Wh1T=Wh1T, WoIT=WoIT,
                b0L=b0L, bh0L=bh0L, bh1L=bh1L,
                bout=bout.astype(np.float64),
                nzh0=nzh0, nzh1=nzh1,
                g_lo=g_lo, g_hi=g_hi, entry=entry, final=final)


def _build(prep):
    nc = bacc.Bacc("TRN2", target_bir_lowering=False, debug=False,
                   num_devices=NCORES)

    def din(name, shape, dt=F32):
        return nc.dram_tensor(name, list(shape), dt, kind="ExternalInput").ap()

    HB = BC // 2                                # 256: batch half per side
    d_w0 = din("w0t", (D, H), MMDT)
    d_w0r = din("w0r", (1, D, H), MMDT)
    d_wh0 = din("wh0t", (P, NB, H), MMDT)
    d_wh1 = din("wh1t", (P, NB, H), MMDT)
    d_wo = din("wot", (P, NB, 2 * D), MMDT)
    d_b0 = din("b0l", (P, NB))
    d_bh0 = din("bh0l", (P, NB))
    d_bh1 = din("bh1l", (P, NB))
    d_z = din("zb", (P, NJ * D), MMDT)          # batch-major [p, j*32+i]
    d_bml = din("bml", (P, NJ * 2 * D))         # bout replicated, interleaved
    d_eye = din("eye", (P, P), MMDT)
    d_out = nc.dram_tensor("out", [P, NJ * D], F32, kind="ExternalOutput").ap()

    bout = prep["bout"]
    nzh0, nzh1 = prep["nzh0"], prep["nzh1"]
    g_lo, g_hi = prep["g_lo"], prep["g_hi"]
    entry, final = prep["entry"], prep["final"]

    def active_at(idx):
        return [b for b in range(NB) if g_lo[b] <= idx - 1 <= g_hi[b]]

    from contextlib import ExitStack
    with tile.TileContext(nc) as tc, ExitStack() as ctx:
        cp = ctx.enter_context(tc.tile_pool(name="const", bufs=1))
        pp = ctx.enter_context(tc.tile_pool(name="psum", bufs=1, space="PSUM"))

        # ---- PSUM: exactly 8 banks (L = batch cols 0:256 -> Act side,
        #      R = cols 256:512 -> DVE side; separate tiles so the dep
        #      tracker lets Act/DVE halves run in parallel) ----
        pz1 = [pp.tile([P, HB], F32, tag=f"pz1{s}", name=f"pz1{s}") for s in "LR"]
        # one bank pair per block PARITY, shared by layers 2 and 3 (z2/z3
        # are chain-sequential), so the two straddle-step blocks overlap.
        # NOTE tags must not collide with the pz1 z1-bank tags.
        pzz = [[pp.tile([P, HB], F32, tag=f"pq{i}{s}", name=f"pq{i}{s}")
                for s in "LR"] for i in range(2)]
        pth = pp.tile([P, NJ, 2 * D], F32, tag="pth", name="pth")
        pmisc = pp.tile([P, 256, 2], F32, tag="pmisc", name="pmisc")
        # pmisc: [:, 0:4, :] pfr (j, mu/ls); partition0 cols 4:132 = xiT
        # chunks (4 x 128 fp16); [0:32, 132:164/164:196] xB-transpose ping/pong

        # ---- SBUF ----
        w0 = cp.tile([D, H], MMDT, tag="w0")
        w0r = cp.tile([1, D, H], MMDT, tag="w0r")
        wh0 = cp.tile([P, NB, H], MMDT, tag="wh0")
        wh1 = cp.tile([P, NB, H], MMDT, tag="wh1")
        wo = cp.tile([P, NB, 2 * D], MMDT, tag="wo")
        eye = cp.tile([P, P], MMDT, tag="eye")
        b0s = cp.tile([P, NB], F32, tag="b0s")
        bh0s = cp.tile([P, NB], F32, tag="bh0s")
        bh1s = cp.tile([P, NB], F32, tag="bh1s")
        zB = cp.tile([P, NJ, D], MMDT, tag="zB")
        xB = cp.tile([P, NJ, D], MMDT, tag="xB")
        xBf = cp.tile([P, NJ * D], F32, tag="xBf")
        xT4 = cp.tile([D, NJ, P], MMDT, tag="xT4")
        thetaS = cp.tile([P, NJ, 2 * D], F32, tag="thetaS")
        bml = cp.tile([P, NJ, 2 * D], F32, tag="bml")
        xiB = cp.tile([P, NJ], MMDT, tag="xiB")
        xiT = cp.tile([1, NJ, P], MMDT, tag="xiT")
        u8 = cp.tile([P, NJ, 2], MMDT, tag="u8")
        es = cp.tile([P, NJ], MMDT, tag="es")
        t2 = cp.tile([P, NJ], MMDT, tag="t2")
        t1L = cp.tile([P, HB], MMDT, tag="t1L")
        t1R = cp.tile([P, HB], MMDT, tag="t1R")
        aL = [[cp.tile([P, HB], MMDT, tag=f"a{l}L{r}", name=f"a{l}L{r}")
               for r in range(NB)] for l in range(3)]
        aR = [[cp.tile([P, HB], MMDT, tag=f"a{l}R{r}", name=f"a{l}R{r}")
               for r in range(NB)] for l in range(3)]
        # S caches, double-buffered by block parity
        S2p = [[cp.tile([P, HB], MMDT, tag=f"S2{s}{i}", name=f"S2{s}{i}")
                for s in "LR"] for i in range(2)]
        S3p = [[cp.tile([P, HB], MMDT, tag=f"S3{s}{i}", name=f"S3{s}{i}")
                for s in "LR"] for i in range(2)]
        z1nL = cp.tile([P, HB], MMDT, tag="z1nL")
        z1nR = cp.tile([P, HB], MMDT, tag="z1nR")

        # ---- DMA in; ~620ns each, serialized on one queue, so order by
        #      first use: everything step 0/1 touches goes first ----
        nc.sync.dma_start(zB[:], d_z)
        nc.sync.dma_start(eye[:], d_eye)
        nc.sync.dma_start(w0r[0:1, 0:8, :], d_w0r[0:1, 0:8, :])
        nc.sync.dma_start(b0s[:], d_b0)
        nc.sync.dma_start(wh0[:, 0, :], d_wh0[:, 0, :])
        nc.sync.dma_start(bh0s[:], d_bh0)
        nc.sync.dma_start(wh1[:, 0, :], d_wh1[:, 0, :])
        nc.sync.dma_start(bh1s[:], d_bh1)
        nc.sync.dma_start(wo[:], d_wo)
        nc.sync.dma_start(thetaS[:], d_bml)      # theta starts as pure bias
        nc.sync.dma_start(bml[:], d_bml)
        nc.sync.dma_start(w0[:], d_w0)
        for i in range(8, D, 8):
            nc.sync.dma_start(w0r[0:1, i:i + 8, :], d_w0r[0:1, i:i + 8, :])
        for c in range(1, NB):
            nc.sync.dma_start(wh0[:, c, :], d_wh0[:, c, :])
            nc.sync.dma_start(wh1[:, c, :], d_wh1[:, c, :])

        nc.vector.memset(xB[:], 0.0)

        xiTv = [pmisc[0:1, 4 + 32 * j:36 + 32 * j, :].bitcast(MMDT)
                for j in range(NJ)]
        xiTfull = pmisc[0:1, 4:132, :].bitcast(MMDT)         # [1,128,4] = 512
        xtt = [pmisc[0:D, 132:164, :].bitcast(MMDT),
               pmisc[0:D, 164:196, :].bitcast(MMDT)]

        def mm(out, lhsT, rhs, start, stop):
            nc.tensor.matmul(out, lhsT, rhs, start=start, stop=stop,
                             skip_group_check=True)

        def relu1(b):
            nc.scalar.activation(aL[0][b][:], pz1[0][:], ActFn.Relu,
                                 bias=b0s[:, b:b + 1], scale=1.0)
            nc.vector.tensor_scalar(aR[0][b][:], pz1[1][:],
                                    b0s[:, b:b + 1], 0.0, AluOp.add, AluOp.max)

        def relu2(b, pz):
            nc.scalar.activation(aL[1][b][:], pz[0][:], ActFn.Relu,
                                 bias=bh0s[:, b:b + 1], scale=1.0)
            nc.vector.tensor_scalar(aR[1][b][:], pz[1][:],
                                    bh0s[:, b:b + 1], 0.0, AluOp.add, AluOp.max)

        def relu3(b, pz):
            nc.scalar.activation(aL[2][b][:], pz[0][:], ActFn.Relu,
                                 bias=bh1s[:, b:b + 1], scale=1.0)
            nc.vector.tensor_scalar(aR[2][b][:], pz[1][:],
                                    bh1s[:, b:b + 1], 0.0, AluOp.add, AluOp.max)

        def xi_transpose():
            for j in range(NJ):
                nc.tensor.transpose(xiTv[j], xiB[:, j:j + 1], eye[:])
            nc.vector.tensor_scalar_add(xiT[:], xiTfull, 0.0)

        def layer_mms(pz, wh, a_in, b, cols, use_S, Ssb):
            """Accumulate one hidden layer for block b into pz (L and R)."""
            if use_S:
                todo = [c for c in cols if c >= b]
                for side in range(2):
                    mm(pz[side][:], eye[:], Ssb[side][:], True, False)
                    for k, c in enumerate(todo):
                        mm(pz[side][:], wh[:, c, b * P:(b + 1) * P],
                           a_in[side][c][:], False, k == len(todo) - 1)
            else:
                for side in range(2):
                    for k, c in enumerate(cols):
                        mm(pz[side][:], wh[:, c, b * P:(b + 1) * P],
                           a_in[side][c][:], k == 0, k == len(cols) - 1)

        # ---- step 0: x_0 = z_0 * exp(bout[D]) + bout[0] ----
        s0 = float(math.exp(bout[D]))
        m0c = float(bout[0])
        nc.vector.tensor_scalar(xiB[:], zB[:, :, 0], s0, m0c,
                                AluOp.mult, AluOp.add)
        nc.gpsimd.tensor_scalar_add(xB[:, :, 0], xiB[:], 0.0)
        xi_transpose()

        S2ready = [False] * NB
        theta_init = [False] * NJ

        for idx in range(1, STOP):
            act_blocks = active_at(idx)
            b_old = act_blocks[0]
            b_new = act_blocks[1] if len(act_blocks) > 1 else None
            ent = [b for b in act_blocks if entry[b] == idx]
            finalizing = [b for b in act_blocks if final[b] == idx]

            # rank-1 z1 for the persisted block (not for entering block;
            # block 0 "enters" at step 1 with a plain start=True rank-1)
            first = (b_old == 0 and idx == 1)
            if entry[b_old] != idx or first:
                mm(pz1[0][:], w0r[0:1, idx - 1, b_old * P:(b_old + 1) * P],
                   xiT[0:1, 0:NJ // 2, :], first, True)
                mm(pz1[1][:], w0r[0:1, idx - 1, b_old * P:(b_old + 1) * P],
                   xiT[0:1, NJ // 2:NJ, :], first, True)
            relu1(b_old)

            # entering block: compute a1 from the prefetched z1next plus a
            # rank-1 in its OWN bank pair, so it does not wait for the old
            # block's relu1 to free pz1. pz1 is restored lazily afterwards
            # (only step idx+1 needs it).
            if b_new is not None:
                pzn = pzz[b_new % 2]
                mm(pzn[0][:], w0r[0:1, idx - 1, b_new * P:(b_new + 1) * P],
                   xiT[0:1, 0:NJ // 2, :], True, True)
                mm(pzn[1][:], w0r[0:1, idx - 1, b_new * P:(b_new + 1) * P],
                   xiT[0:1, NJ // 2:NJ, :], True, True)
                nc.vector.scalar_tensor_tensor(
                    t1L[:], pzn[0][:], b0s[:, b_new:b_new + 1], z1nL[:],
                    AluOp.add, AluOp.add)
                nc.vector.scalar_tensor_tensor(
                    t1R[:], pzn[1][:], b0s[:, b_new:b_new + 1], z1nR[:],
                    AluOp.add, AluOp.add)
                nc.scalar.activation(aL[0][b_new][:], t1L[:], ActFn.Relu)
                nc.vector.tensor_scalar_max(aR[0][b_new][:], t1R[:], 0.0)
                # lazy pz1 hand-off for subsequent steps
                mm(pz1[0][:], eye[:], z1nL[:], True, False)
                mm(pz1[0][:], w0r[0:1, idx - 1, b_new * P:(b_new + 1) * P],
                   xiT[0:1, 0:NJ // 2, :], False, True)
                mm(pz1[1][:], eye[:], z1nR[:], True, False)
                mm(pz1[1][:], w0r[0:1, idx - 1, b_new * P:(b_new + 1) * P],
                   xiT[0:1, NJ // 2:NJ, :], False, True)

            # -- layer 2 --
            a1 = (aL[0], aR[0])
            a2 = (aL[1], aR[1])
            a3 = (aL[2], aR[2])
            def enter_layer(pz, wh, a_in, b, cols, Sp):
                """Entering block's layer: restore prefetched S' (c<=b-2),
                then the c=b-1 term and the diagonal. Correct because
                a1/a2[b-1] are FINAL this step."""
                for side in range(2):
                    got = False
                    if any(c <= b - 2 for c in cols):
                        mm(pz[side][:], eye[:], Sp[side][:], True, False)
                        got = True
                    for c in cols:
                        if c == b - 1:
                            mm(pz[side][:], wh[:, c, b * P:(b + 1) * P],
                               a_in[side][c][:], not got, False)
                            got = True
                    mm(pz[side][:], wh[:, b, b * P:(b + 1) * P],
                       a_in[side][b][:], False, True)

            # bank assignment: steady steps alternate z2/z3 across the two
            # pairs (restores run early, off-chain); straddle steps give each
            # block its own dedicated pair so the two pipelines overlap.
            z2o = pzz[b_old % 2]
            z3o = pzz[b_old % 2] if b_new is not None else pzz[(b_old + 1) % 2]
            cols2_old = [c for c in range(NB)
                         if nzh0[b_old, c] and g_lo[c] <= idx - 1]
            layer_mms(z2o, wh0, a1, b_old, cols2_old,
                      S2ready[b_old], S2p[b_old % 2])
            relu2(b_old, z2o)
            if b_new is not None:
                cols2_new = [c for c in range(NB)
                             if nzh0[b_new, c] and g_lo[c] <= idx - 1]
                enter_layer(pzz[b_new % 2], wh0, a1, b_new, cols2_new,
                            S2p[b_new % 2])
                relu2(b_new, pzz[b_new % 2])

            # -- layer 3 (old block may need a2[b_new]: emitted after) --
            cols3_old = [c for c in range(NB)
                         if nzh1[b_old, c] and g_lo[c] <= idx - 1]
            layer_mms(z3o, wh1, a2, b_old, cols3_old,
                      S2ready[b_old], S3p[b_old % 2])
            relu3(b_old, z3o)
            if b_new is not None:
                cols3_new = [c for c in range(NB)
                             if nzh1[b_new, c] and g_lo[c] <= idx - 1]
                enter_layer(pzz[b_new % 2], wh1, a2, b_new, cols3_new,
                            S3p[b_new % 2])
                relu3(b_new, pzz[b_new % 2])

            # -- frontier output contribution (batch-major, N=2); theta (with
            #    folded biases) is accumulated INTO the same psum by early
            #    identity-matmuls, so exp reads psum directly and the tail's
            #    DVE add disappears --
            for j in range(NJ):
                side, jj = (0, j) if j < NJ // 2 else (1, j - NJ // 2)
                for k, b in enumerate(act_blocks):
                    mm(pmisc[:, j, 0:2],
                       a3[side][b][:, jj * P:(jj + 1) * P],
                       wo[:, b, 2 * idx:2 * idx + 2],
                       k == 0, k == len(act_blocks) - 1)

            # -- tail --
            nc.vector.tensor_tensor(u8[:], pmisc[:, 0:NJ, :],
                                    thetaS[:, :, 2 * idx:2 * idx + 2],
                                    AluOp.add)
            nc.scalar.activation(es[:], u8[:, :, 1], ActFn.Exp)
            nc.gpsimd.tensor_tensor(t2[:], es[:], zB[:, :, idx], AluOp.mult)
            nc.gpsimd.tensor_tensor(xiB[:], t2[:], u8[:, :, 0], AluOp.add)
            if idx < STOP - 1:
                xi_transpose()
            nc.gpsimd.tensor_scalar_add(xB[:, :, idx], xiB[:], 0.0)

            # -- finalize theta (after tail read of pmisc/thetaS) --
            for b in finalizing:
                if idx >= STOP - 1:
                    continue
                for j in range(NJ):
                    side, jj = (0, j) if j < NJ // 2 else (1, j - NJ // 2)
                    mm(pth[:, j, :],
                       a3[side][b][:, jj * P:(jj + 1) * P],
                       wo[:, b, :],
                       not theta_init[j], True)
                    theta_init[j] = True
                nc.vector.tensor_tensor(thetaS[:], pth[:], bml[:], AluOp.add)

            # -- full S2/S3 cache for the lone active block, one step after
            #    entry; runs in the free bank pairs, hidden in chain slack --
            b = b_old
            if b > 0 and not S2ready[b] and idx == entry[b] + 1 \
                    and idx < final[b]:
                cc2 = [c for c in range(NB) if nzh0[b, c] and c < b]
                cc3 = [c for c in range(NB) if nzh1[b, c] and c < b]
                scr2 = pzz[b % 2]          # free after this step's relu2
                scr3 = pzz[(b + 1) % 2]    # free after this step's relu3
                for side in range(2):
                    for k, c in enumerate(cc2):
                        mm(scr2[side][:], wh0[:, c, b * P:(b + 1) * P],
                           a1[side][c][:], k == 0, k == len(cc2) - 1)
                nc.scalar.copy(S2p[b % 2][0][:], scr2[0][:])
                nc.vector.tensor_scalar_add(S2p[b % 2][1][:], scr2[1][:], 0.0)
                for side in range(2):
                    for k, c in enumerate(cc3):
                        mm(scr3[side][:], wh1[:, c, b * P:(b + 1) * P],
                           a2[side][c][:], k == 0, k == len(cc3) - 1)
                nc.scalar.copy(S3p[b % 2][0][:], scr3[0][:])
                nc.vector.tensor_scalar_add(S3p[b % 2][1][:], scr3[1][:], 0.0)
                S2ready[b] = True
            if final[b_old] == idx:
                S2ready[b_old] = False

            # -- prefetch for the block entering next step: z1 and the
            #    S' partial sums over already-final inputs (c <= bb-2) --
            pre = [bb for bb in range(1, NB) if entry[bb] == idx + 1]
            for bb in pre:
                for j in range(NJ):
                    pg = xtt[j % 2]
                    nc.tensor.transpose(pg, xB[:, j, :], eye[:])
                    nc.vector.tensor_scalar_add(xT4[:, j, :], pg, 0.0)
                scr = None
                scr = pzz[bb % 2]
                mm(scr[0][:], w0[:, bb * P:(bb + 1) * P],
                   xT4[:, 0:NJ // 2, :], True, True)
                mm(scr[1][:], w0[:, bb * P:(bb + 1) * P],
                   xT4[:, NJ // 2:NJ, :], True, True)
                nc.scalar.copy(z1nL[:], scr[0][:])
                nc.vector.tensor_scalar_add(z1nR[:], scr[1][:], 0.0)
                cc2 = [c for c in range(NB) if nzh0[bb, c] and c <= bb - 2]
                cc3 = [c for c in range(NB) if nzh1[bb, c] and c <= bb - 2]
                if cc2:
                    scr2 = pzz[(bb + 1) % 2]
                    for side in range(2):
                        for k, c in enumerate(cc2):
                            mm(scr2[side][:], wh0[:, c, bb * P:(bb + 1) * P],
                               a1[side][c][:], k == 0, k == len(cc2) - 1)
                    nc.scalar.copy(S2p[bb % 2][0][:], scr2[0][:])
                    nc.vector.tensor_scalar_add(S2p[bb % 2][1][:],
                                                scr2[1][:], 0.0)
                if cc3:
                    for side in range(2):
                        for k, c in enumerate(cc3):
                            mm(scr[side][:], wh1[:, c, bb * P:(bb + 1) * P],
                               a2[side][c][:], k == 0, k == len(cc3) - 1)
                    nc.scalar.copy(S3p[bb % 2][0][:], scr[0][:])
                    nc.vector.tensor_scalar_add(S3p[bb % 2][1][:],
                                                scr[1][:], 0.0)

        # ---- output ----
        nc.scalar.copy(xBf[:], xB[:])
        nc.sync.dma_start(d_out, xBf[:])

    nc.compile()
    return nc


_CACHE = {}


def _get_program(prep):
    key = (DTYPE, STOP, hashlib.md5(prep["bout"].tobytes()).hexdigest())
    if key not in _CACHE:
        _CACHE[key] = _build(prep)
    return _CACHE[key]


def _run(inputs, trace=False):
    z = np.asarray(inputs["z"], dtype=np.float32)
    prep = _host_prep(np.asarray(inputs["W0"], np.float32),
                      np.asarray(inputs["b0"], np.float32),
                      np.asarray(inputs["Wh"], np.float32),
                      np.asarray(inputs["bh"], np.float32),
                      np.asarray(inputs["Wout"], np.float32),
                      np.asarray(inputs["bout"], np.float32))
    nc = _get_program(prep)

    eye = np.eye(P, dtype=NPDT)
    bout32 = prep["bout"].astype(np.float32)
    bml1 = np.empty(2 * D, dtype=np.float32)
    bml1[0::2] = bout32[:D]
    bml1[1::2] = bout32[D:]
    bml = np.ascontiguousarray(
        np.broadcast_to(np.tile(bml1, NJ), (P, NJ * 2 * D))).astype(np.float32)
    in_maps = []
    for c in range(NCORES):
        zs = z[c * BC:(c + 1) * BC, :]                     # [512, 32]
        # batch-major: [p, j, i] = z[j*128+p, i]
        zb = np.ascontiguousarray(
            zs.reshape(NJ, P, D).transpose(1, 0, 2).reshape(P, NJ * D)
        ).astype(NPDT)
        in_maps.append({
            "w0t": prep["W0T"], "wh0t": prep["Wh0T"], "wh1t": prep["Wh1T"],
            "wot": prep["WoIT"], "b0l": prep["b0L"], "bh0l": prep["bh0L"],
            "bh1l": prep["bh1L"], "zb": zb, "eye": eye,
            "w0r": prep["W0T"].reshape(1, D, H), "bml": bml,
        })

    res = run_bass_kernel_spmd(nc, in_maps, core_ids=list(range(NCORES)),
                               trace=trace)
    out = np.empty((B, D), dtype=np.float32)
    for c in range(NCORES):
        buf = res.results[c]["out"]                        # [128, 128]
        out[c * BC:(c + 1) * BC, :] = (
            buf.reshape(P, NJ, D).transpose(1, 0, 2).reshape(BC, D))
    return out, res


def kernel(**inputs):
    out, _ = _run(inputs, trace=False)
    return out


# revision 55
# speedup vs baseline: 1.0342x; 1.0342x over previous
"""MADE autoregressive sampler on 8 TRN2 NeuronCores — incremental frontier.

Strategy (vs. the full-recompute baseline):
- Data-parallel over batch: B=4096 -> 512 rows per core; weights replicated.
- Degree-sort hidden units. In MADE, a hidden unit's activation is FINAL once
  x columns 0..deg are set, so per AR step only the 1-2 "frontier" blocks
  (those containing degree idx-1) need recomputation. Everything else is
  computed once and cached:
    * z1 (layer-1 preact) kept in PSUM, updated by a rank-1 matmul per step.
    * S2/S3 = frozen off-diagonal partial sums per frontier block, cached in
      SBUF and restored into PSUM each step (then diag matmul accumulates).
    * theta (output-layer contributions of finalized blocks) accumulates in
      one PSUM bank, in batch-major chunk layout so the per-step tail ops are
      [128, 4] instead of [1, 512].
- fp16 operands everywhere (fp32 PSUM accumulation).
- Elementwise load spread across Scalar(Act)/Vector(DVE)/Pool(gpsimd).
"""

import os
import sys
import math
import hashlib
import numpy as np

for _p in ("/opt/trn_rl_repo", "/opt/pypackages"):
    if _p not in sys.path:
        sys.path.insert(0, _p)

import concourse.bass as bass
import concourse.tile as tile
from concourse import bacc
from concourse import mybir
from concourse.bass_utils import run_bass_kernel_spmd

D, H, L, B = 32, 1024, 2, 4096
NCORES = 8
BC = B // NCORES          # 512 batch rows per core
P = 128
NB = H // P               # 8 hidden blocks
NJ = BC // P              # 4 batch chunks of 128
F32 = mybir.dt.float32
F16 = mybir.dt.float16

DTYPE = os.environ.get("MADE_DTYPE", "fp16")
MMDT = {"fp16": mybir.dt.float16, "bf16": mybir.dt.bfloat16,
        "f32r": mybir.dt.float32r}[DTYPE]
NPDT = {"fp16": np.float16, "bf16": np.float32, "f32r": np.float32}[DTYPE]
STOP = int(os.environ.get("MADE_STOP", "32"))

AluOp = mybir.AluOpType
ActFn = mybir.ActivationFunctionType


def _schedule():
    """Static per-step schedule from the degree structure."""
    d_hid = np.arange(H) % (D - 1)
    perm = np.argsort(d_hid, kind="stable")
    ds = d_hid[perm]
    g_lo = [int(ds[P * b]) for b in range(NB)]
    g_hi = [int(ds[P * b + P - 1]) for b in range(NB)]
    entry = [g_lo[b] + 1 for b in range(NB)]
    final = [g_hi[b] + 1 for b in range(NB)]
    return perm, ds, g_lo, g_hi, entry, final


def _host_prep(W0, b0, Wh, bh, Wout, bout):
    d_in = np.arange(D)
    d_hid = np.arange(H) % (D - 1)
    d_out = np.arange(D) - 1
    m0 = (d_hid[:, None] >= d_in[None, :]).astype(np.float32)
    mh = (d_hid[:, None] >= d_hid[None, :]).astype(np.float32)
    mo = (d_out[:, None] >= d_hid[None, :]).astype(np.float32)
    mo = np.concatenate([mo, mo], axis=0)

    perm, ds, g_lo, g_hi, entry, final = _schedule()

    W0p = (m0 * W0)[perm]                     # [H, D]
    Wh0p = (mh * Wh[0])[perm][:, perm]        # [H, H] (out, in)
    Wh1p = (mh * Wh[1])[perm][:, perm]
    Wop = (mo * Wout)[:, perm]                # [2D, H]
    b0p = b0[perm]
    bh0p = bh[0][perm]
    bh1p = bh[1][perm]

    # lhsT layouts
    W0T = np.ascontiguousarray(W0p.T).astype(NPDT)          # [32, H]
    Wh0T = np.ascontiguousarray(
        Wh0p.T.reshape(NB, P, H).transpose(1, 0, 2)).astype(NPDT)  # [128, NB, H]
    Wh1T = np.ascontiguousarray(
        Wh1p.T.reshape(NB, P, H).transpose(1, 0, 2)).astype(NPDT)
    # interleaved output weights: col 2i = mu_i, col 2i+1 = ls_i
    WoI = np.empty((H, 2 * D), dtype=np.float32)
    WoI[:, 0::2] = Wop[:D, :].T
    WoI[:, 1::2] = Wop[D:, :].T
    WoIT = np.ascontiguousarray(
        WoI.reshape(NB, P, 2 * D).transpose(1, 0, 2)).astype(NPDT)  # [128, NB, 64]

    b0L = np.ascontiguousarray(b0p.reshape(NB, P).T).astype(np.float32)
    bh0L = np.ascontiguousarray(bh0p.reshape(NB, P).T).astype(np.float32)
    bh1L = np.ascontiguousarray(bh1p.reshape(NB, P).T).astype(np.float32)

    nzh0 = np.zeros((NB, NB), dtype=bool)
    nzh1 = np.zeros((NB, NB), dtype=bool)
    for r in range(NB):
        for c in range(NB):
            nzh0[r, c] = bool(np.any(Wh0p[r * P:(r + 1) * P, c * P:(c + 1) * P]))
            nzh1[r, c] = bool(np.any(Wh1p[r * P:(r + 1) * P, c * P:(c + 1) * P]))

    return dict(W0T=W0T, Wh0T=Wh0T, Wh1T=Wh1T, WoIT=WoIT,
                b0L=b0L, bh0L=bh0L, bh1L=bh1L,
                bout=bout.astype(np.float64),
                nzh0=nzh0, nzh1=nzh1,
                g_lo=g_lo, g_hi=g_hi, entry=entry, final=final)


def _build(prep):
    nc = bacc.Bacc("TRN2", target_bir_lowering=False, debug=False,
                   num_devices=NCORES)

    def din(name, shape, dt=F32):
        return nc.dram_tensor(name, list(shape), dt, kind="ExternalInput").ap()

    HB = BC // 2                                # 256: batch half per side
    d_w0 = din("w0t", (D, H), MMDT)
    d_w0r = din("w0r", (1, D, H), MMDT)
    d_wh0 = din("wh0t", (P, NB, H), MMDT)
    d_wh1 = din("wh1t", (P, NB, H), MMDT)
    d_wo = din("wot", (P, NB, 2 * D), MMDT)
    d_b0 = din("b0l", (P, NB))
    d_bh0 = din("bh0l", (P, NB))
    d_bh1 = din("bh1l", (P, NB))
    d_z = din("zb", (P, NJ * D), MMDT)          # batch-major [p, j*32+i]
    d_bml = din("bml", (P, NJ * 2 * D))         # bout replicated, interleaved
    d_eye = din("eye", (P, P), MMDT)
    d_out = nc.dram_tensor("out", [P, NJ * D], F32, kind="ExternalOutput").ap()

    bout = prep["bout"]
    nzh0, nzh1 = prep["nzh0"], prep["nzh1"]
    g_lo, g_hi = prep["g_lo"], prep["g_hi"]
    entry, final = prep["entry"], prep["final"]

    def active_at(idx):
        return [b for b in range(NB) if g_lo[b] <= idx - 1 <= g_hi[b]]

    from contextlib import ExitStack
    with tile.TileContext(nc) as tc, ExitStack() as ctx:
        cp = ctx.enter_context(tc.tile_pool(name="const", bufs=1))
        pp = ctx.enter_context(tc.tile_pool(name="psum", bufs=1, space="PSUM"))

        # ---- PSUM: exactly 8 banks (L = batch cols 0:256 -> Act side,
        #      R = cols 256:512 -> DVE side; separate tiles so the dep
        #      tracker lets Act/DVE halves run in parallel) ----
        pz1 = [pp.tile([P, HB], F32, tag=f"pz1{s}", name=f"pz1{s}") for s in "LR"]
        # one bank pair per block PARITY, shared by layers 2 and 3 (z2/z3
        # are chain-sequential), so the two straddle-step blocks overlap.
        # NOTE tags must not collide with the pz1 z1-bank tags.
        pzz = [[pp.tile([P, HB], F32, tag=f"pq{i}{s}", name=f"pq{i}{s}")
                for s in "LR"] for i in range(2)]
        pth = pp.tile([P, NJ, 2 * D], F32, tag="pth", name="pth")
        pmisc = pp.tile([P, 256, 2], F32, tag="pmisc", name="pmisc")
        # pmisc: [:, 0:4, :] pfr (j, mu/ls); partition0 cols 4:132 = xiT
        # chunks (4 x 128 fp16); [0:32, 132:164/164:196] xB-transpose ping/pong

        # ---- SBUF ----
        w0 = cp.tile([D, H], MMDT, tag="w0")
        w0r = cp.tile([1, D, H], MMDT, tag="w0r")
        wh0 = cp.tile([P, NB, H], MMDT, tag="wh0")
        wh1 = cp.tile([P, NB, H], MMDT, tag="wh1")
        wo = cp.tile([P, NB, 2 * D], MMDT, tag="wo")
        eye = cp.tile([P, P], MMDT, tag="eye")
        b0s = cp.tile([P, NB], F32, tag="b0s")
        bh0s = cp.tile([P, NB], F32, tag="bh0s")
        bh1s = cp.tile([P, NB], F32, tag="bh1s")
        zB = cp.tile([P, NJ, D], MMDT, tag="zB")
        xB = cp.tile([P, NJ, D], MMDT, tag="xB")
        xBf = cp.tile([P, NJ * D], F32, tag="xBf")
        xT4 = cp.tile([D, NJ, P], MMDT, tag="xT4")
        thetaS = cp.tile([P, NJ, 2 * D], F32, tag="thetaS")
        bml = cp.tile([P, NJ, 2 * D], F32, tag="bml")
        xiB = cp.tile([P, NJ], MMDT, tag="xiB")
        xiT = cp.tile([1, NJ, P], MMDT, tag="xiT")
        u8 = cp.tile([P, NJ, 2], MMDT, tag="u8")
        es = cp.tile([P, NJ], MMDT, tag="es")
        t2 = cp.tile([P, NJ], MMDT, tag="t2")
        aL = [[cp.tile([P, HB], MMDT, tag=f"a{l}L{r}", name=f"a{l}L{r}")
               for r in range(NB)] for l in range(3)]
        aR = [[cp.tile([P, HB], MMDT, tag=f"a{l}R{r}", name=f"a{l}R{r}")
               for r in range(NB)] for l in range(3)]
        # S caches, double-buffered by block parity
        S2p = [[cp.tile([P, HB], MMDT, tag=f"S2{s}{i}", name=f"S2{s}{i}")
                for s in "LR"] for i in range(2)]
        S3p = [[cp.tile([P, HB], MMDT, tag=f"S3{s}{i}", name=f"S3{s}{i}")
                for s in "LR"] for i in range(2)]
        z1nL = cp.tile([P, HB], MMDT, tag="z1nL")
        z1nR = cp.tile([P, HB], MMDT, tag="z1nR")

        # ---- DMA in; ~620ns each, serialized on one queue, so order by
        #      first use: everything step 0/1 touches goes first ----
        nc.sync.dma_start(zB[:], d_z)
        nc.sync.dma_start(eye[:], d_eye)
        nc.sync.dma_start(w0r[0:1, 0:8, :], d_w0r[0:1, 0:8, :])
        nc.sync.dma_start(b0s[:], d_b0)
        nc.sync.dma_start(wh0[:, 0, :], d_wh0[:, 0, :])
        nc.sync.dma_start(bh0s[:], d_bh0)
        nc.sync.dma_start(wh1[:, 0, :], d_wh1[:, 0, :])
        nc.sync.dma_start(bh1s[:], d_bh1)
        nc.sync.dma_start(wo[:], d_wo)
        nc.sync.dma_start(thetaS[:], d_bml)      # theta starts as pure bias
        nc.sync.dma_start(bml[:], d_bml)
        nc.sync.dma_start(w0[:], d_w0)
        for i in range(8, D, 8):
            nc.sync.dma_start(w0r[0:1, i:i + 8, :], d_w0r[0:1, i:i + 8, :])
        for c in range(1, NB):
            nc.sync.dma_start(wh0[:, c, :], d_wh0[:, c, :])
            nc.sync.dma_start(wh1[:, c, :], d_wh1[:, c, :])

        nc.vector.memset(xB[:], 0.0)

        xiTv = [pmisc[0:1, 4 + 32 * j:36 + 32 * j, :].bitcast(MMDT)
                for j in range(NJ)]
        xiTfull = pmisc[0:1, 4:132, :].bitcast(MMDT)         # [1,128,4] = 512
        xtt = [pmisc[0:D, 132:164, :].bitcast(MMDT),
               pmisc[0:D, 164:196, :].bitcast(MMDT)]

        def mm(out, lhsT, rhs, start, stop):
            nc.tensor.matmul(out, lhsT, rhs, start=start, stop=stop,
                             skip_group_check=True)

        def relu1(b):
            nc.scalar.activation(aL[0][b][:], pz1[0][:], ActFn.Relu,
                                 bias=b0s[:, b:b + 1], scale=1.0)
            nc.vector.tensor_scalar(aR[0][b][:], pz1[1][:],
                                    b0s[:, b:b + 1], 0.0, AluOp.add, AluOp.max)

        def relu2(b, pz):
            nc.scalar.activation(aL[1][b][:], pz[0][:], ActFn.Relu,
                                 bias=bh0s[:, b:b + 1], scale=1.0)
            nc.vector.tensor_scalar(aR[1][b][:], pz[1][:],
                                    bh0s[:, b:b + 1], 0.0, AluOp.add, AluOp.max)

        def relu3(b, pz):
            nc.scalar.activation(aL[2][b][:], pz[0][:], ActFn.Relu,
                                 bias=bh1s[:, b:b + 1], scale=1.0)
            nc.vector.tensor_scalar(aR[2][b][:], pz[1][:],
                                    bh1s[:, b:b + 1], 0.0, AluOp.add, AluOp.max)

        def xi_transpose():
            for j in range(NJ):
                nc.tensor.transpose(xiTv[j], xiB[:, j:j + 1], eye[:])
            nc.vector.tensor_scalar_add(xiT[:], xiTfull, 0.0)

        def layer_mms(pz, wh, a_in, b, cols, use_S, Ssb):
            """Accumulate one hidden layer for block b into pz (L and R)."""
            if use_S:
                todo = [c for c in cols if c >= b]
                for side in range(2):
                    mm(pz[side][:], eye[:], Ssb[side][:], True, False)
                    for k, c in enumerate(todo):
                        mm(pz[side][:], wh[:, c, b * P:(b + 1) * P],
                           a_in[side][c][:], False, k == len(todo) - 1)
            else:
                for side in range(2):
                    for k, c in enumerate(cols):
                        mm(pz[side][:], wh[:, c, b * P:(b + 1) * P],
                           a_in[side][c][:], k == 0, k == len(cols) - 1)

        # ---- step 0: x_0 = z_0 * exp(bout[D]) + bout[0] ----
        s0 = float(math.exp(bout[D]))
        m0c = float(bout[0])
        nc.vector.tensor_scalar(xiB[:], zB[:, :, 0], s0, m0c,
                                AluOp.mult, AluOp.add)
        nc.gpsimd.tensor_scalar_add(xB[:, :, 0], xiB[:], 0.0)
        xi_transpose()

        S2ready = [False] * NB
        theta_init = [False] * NJ

        for idx in range(1, STOP):
            act_blocks = active_at(idx)
            b_old = act_blocks[0]
            b_new = act_blocks[1] if len(act_blocks) > 1 else None
            ent = [b for b in act_blocks if entry[b] == idx]
            finalizing = [b for b in act_blocks if final[b] == idx]

            # rank-1 z1 for the persisted block (not for entering block;
            # block 0 "enters" at step 1 with a plain start=True rank-1)
            first = (b_old == 0 and idx == 1)
            if entry[b_old] != idx or first:
                mm(pz1[0][:], w0r[0:1, idx - 1, b_old * P:(b_old + 1) * P],
                   xiT[0:1, 0:NJ // 2, :], first, True)
                mm(pz1[1][:], w0r[0:1, idx - 1, b_old * P:(b_old + 1) * P],
                   xiT[0:1, NJ // 2:NJ, :], first, True)
            relu1(b_old)

            # entering block: overwrite pz1 after old relu1 read
            if b_new is not None:
                mm(pz1[0][:], eye[:], z1nL[:], True, False)
                mm(pz1[0][:], w0r[0:1, idx - 1, b_new * P:(b_new + 1) * P],
                   xiT[0:1, 0:NJ // 2, :], False, True)
                mm(pz1[1][:], eye[:], z1nR[:], True, False)
                mm(pz1[1][:], w0r[0:1, idx - 1, b_new * P:(b_new + 1) * P],
                   xiT[0:1, NJ // 2:NJ, :], False, True)
                relu1(b_new)

            # -- layer 2 --
            a1 = (aL[0], aR[0])
            a2 = (aL[1], aR[1])
            a3 = (aL[2], aR[2])
            def enter_layer(pz, wh, a_in, b, cols, Sp):
                """Entering block's layer: restore prefetched S' (c<=b-2),
                then the c=b-1 term and the diagonal. Correct because
                a1/a2[b-1] are FINAL this step."""
                for side in range(2):
                    got = False
                    if any(c <= b - 2 for c in cols):
                        mm(pz[side][:], eye[:], Sp[side][:], True, False)
                        got = True
                    for c in cols:
                        if c == b - 1:
                            mm(pz[side][:], wh[:, c, b * P:(b + 1) * P],
                               a_in[side][c][:], not got, False)
                            got = True
                    mm(pz[side][:], wh[:, b, b * P:(b + 1) * P],
                       a_in[side][b][:], False, True)

            # bank assignment: steady steps alternate z2/z3 across the two
            # pairs (restores run early, off-chain); straddle steps give each
            # block its own dedicated pair so the two pipelines overlap.
            z2o = pzz[b_old % 2]
            z3o = pzz[b_old % 2] if b_new is not None else pzz[(b_old + 1) % 2]
            cols2_old = [c for c in range(NB)
                         if nzh0[b_old, c] and g_lo[c] <= idx - 1]
            layer_mms(z2o, wh0, a1, b_old, cols2_old,
                      S2ready[b_old], S2p[b_old % 2])
            relu2(b_old, z2o)
            if b_new is not None:
                cols2_new = [c for c in range(NB)
                             if nzh0[b_new, c] and g_lo[c] <= idx - 1]
                enter_layer(pzz[b_new % 2], wh0, a1, b_new, cols2_new,
                            S2p[b_new % 2])
                relu2(b_new, pzz[b_new % 2])

            # -- layer 3 (old block may need a2[b_new]: emitted after) --
            cols3_old = [c for c in range(NB)
                         if nzh1[b_old, c] and g_lo[c] <= idx - 1]
            layer_mms(z3o, wh1, a2, b_old, cols3_old,
                      S2ready[b_old], S3p[b_old % 2])
            relu3(b_old, z3o)
            if b_new is not None:
                cols3_new = [c for c in range(NB)
                             if nzh1[b_new, c] and g_lo[c] <= idx - 1]
                enter_layer(pzz[b_new % 2], wh1, a2, b_new, cols3_new,
                            S3p[b_new % 2])
                relu3(b_new, pzz[b_new % 2])

            # -- frontier output contribution (batch-major, N=2); theta (with
            #    folded biases) is accumulated INTO the same psum by early
            #    identity-matmuls, so exp reads psum directly and the tail's
            #    DVE add disappears --
            for j in range(NJ):
                side, jj = (0, j) if j < NJ // 2 else (1, j - NJ // 2)
                for k, b in enumerate(act_blocks):
                    mm(pmisc[:, j, 0:2],
                       a3[side][b][:, jj * P:(jj + 1) * P],
                       wo[:, b, 2 * idx:2 * idx + 2],
                       k == 0, k == len(act_blocks) - 1)

            # -- tail --
            nc.vector.tensor_tensor(u8[:], pmisc[:, 0:NJ, :],
                                    thetaS[:, :, 2 * idx:2 * idx + 2],
                                    AluOp.add)
            nc.scalar.activation(es[:], u8[:, :, 1], ActFn.Exp)
            nc.gpsimd.tensor_tensor(t2[:], es[:], zB[:, :, idx], AluOp.mult)
            nc.gpsimd.tensor_tensor(xiB[:], t2[:], u8[:, :, 0], AluOp.add)
            if idx < STOP - 1:
                xi_transpose()
            nc.gpsimd.tensor_scalar_add(xB[:, :, idx], xiB[:], 0.0)

            # -- finalize theta (after tail read of pmisc/thetaS) --
            for b in finalizing:
                if idx >= STOP - 1:
                    continue
                for j in range(NJ):
                    side, jj = (0, j) if j < NJ // 2 else (1, j - NJ // 2)
                    mm(pth[:, j, :],
                       a3[side][b][:, jj * P:(jj + 1) * P],
                       wo[:, b, :],
                       not theta_init[j], True)
                    theta_init[j] = True
                nc.vector.tensor_tensor(thetaS[:], pth[:], bml[:], AluOp.add)

            # -- full S2/S3 cache for the lone active block, one step after
            #    entry; runs in the free bank pairs, hidden in chain slack --
            b = b_old
            if b > 0 and not S2ready[b] and idx == entry[b] + 1 \
                    and idx < final[b]:
                cc2 = [c for c in range(NB) if nzh0[b, c] and c < b]
                cc3 = [c for c in range(NB) if nzh1[b, c] and c < b]
                scr2 = pzz[b % 2]          # free after this step's relu2
                scr3 = pzz[(b + 1) % 2]    # free after this step's relu3
                for side in range(2):
                    for k, c in enumerate(cc2):
                        mm(scr2[side][:], wh0[:, c, b * P:(b + 1) * P],
                           a1[side][c][:], k == 0, k == len(cc2) - 1)
                nc.scalar.copy(S2p[b % 2][0][:], scr2[0][:])
                nc.vector.tensor_scalar_add(S2p[b % 2][1][:], scr2[1][:], 0.0)
                for side in range(2):
                    for k, c in enumerate(cc3):
                        mm(scr3[side][:], wh1[:, c, b * P:(b + 1) * P],
                           a2[side][c][:], k == 0, k == len(cc3) - 1)
                nc.scalar.copy(S3p[b % 2][0][:], scr3[0][:])
                nc.vector.tensor_scalar_add(S3p[b % 2][1][:], scr3[1][:], 0.0)
                S2ready[b] = True
            if final[b_old] == idx:
                S2ready[b_old] = False

            # -- prefetch for the block entering next step: z1 and the
            #    S' partial sums over already-final inputs (c <= bb-2) --
            pre = [bb for bb in range(1, NB) if entry[bb] == idx + 1]
            for bb in pre:
                for j in range(NJ):
                    pg = xtt[j % 2]
                    nc.tensor.transpose(pg, xB[:, j, :], eye[:])
                    nc.vector.tensor_scalar_add(xT4[:, j, :], pg, 0.0)
                scr = None
                scr = pzz[bb % 2]
                mm(scr[0][:], w0[:, bb * P:(bb + 1) * P],
                   xT4[:, 0:NJ // 2, :], True, True)
                mm(scr[1][:], w0[:, bb * P:(bb + 1) * P],
                   xT4[:, NJ // 2:NJ, :], True, True)
                nc.scalar.copy(z1nL[:], scr[0][:])
                nc.vector.tensor_scalar_add(z1nR[:], scr[1][:], 0.0)
                cc2 = [c for c in range(NB) if nzh0[bb, c] and c <= bb - 2]
                cc3 = [c for c in range(NB) if nzh1[bb, c] and c <= bb - 2]
                if cc2:
                    scr2 = pzz[(bb + 1) % 2]
                    for side in range(2):
                        for k, c in enumerate(cc2):
                            mm(scr2[side][:], wh0[:, c, bb * P:(bb + 1) * P],
                               a1[side][c][:], k == 0, k == len(cc2) - 1)
                    nc.scalar.copy(S2p[bb % 2][0][:], scr2[0][:])
                    nc.vector.tensor_scalar_add(S2p[bb % 2][1][:],
                                                scr2[1][:], 0.0)
                if cc3:
                    for side in range(2):
                        for k, c in enumerate(cc3):
                            mm(scr[side][:], wh1[:, c, bb * P:(bb + 1) * P],
                               a2[side][c][:], k == 0, k == len(cc3) - 1)
                    nc.scalar.copy(S3p[bb % 2][0][:], scr[0][:])
                    nc.vector.tensor_scalar_add(S3p[bb % 2][1][:],
                                                scr[1][:], 0.0)

        # ---- output ----
        nc.scalar.copy(xBf[:], xB[:])
        nc.sync.dma_start(d_out, xBf[:])

    nc.compile()
    return nc


_CACHE = {}


def _get_program(prep):
    key = (DTYPE, STOP, hashlib.md5(prep["bout"].tobytes()).hexdigest())
    if key not in _CACHE:
        _CACHE[key] = _build(prep)
    return _CACHE[key]


def _run(inputs, trace=False):
    z = np.asarray(inputs["z"], dtype=np.float32)
    prep = _host_prep(np.asarray(inputs["W0"], np.float32),
                      np.asarray(inputs["b0"], np.float32),
                      np.asarray(inputs["Wh"], np.float32),
                      np.asarray(inputs["bh"], np.float32),
                      np.asarray(inputs["Wout"], np.float32),
                      np.asarray(inputs["bout"], np.float32))
    nc = _get_program(prep)

    eye = np.eye(P, dtype=NPDT)
    bout32 = prep["bout"].astype(np.float32)
    bml1 = np.empty(2 * D, dtype=np.float32)
    bml1[0::2] = bout32[:D]
    bml1[1::2] = bout32[D:]
    bml = np.ascontiguousarray(
        np.broadcast_to(np.tile(bml1, NJ), (P, NJ * 2 * D))).astype(np.float32)
    in_maps = []
    for c in range(NCORES):
        zs = z[c * BC:(c + 1) * BC, :]                     # [512, 32]
        # batch-major: [p, j, i] = z[j*128+p, i]
        zb = np.ascontiguousarray(
            zs.reshape(NJ, P, D).transpose(1, 0, 2).reshape(P, NJ * D)
        ).astype(NPDT)
        in_maps.append({
            "w0t": prep["W0T"], "wh0t": prep["Wh0T"], "wh1t": prep["Wh1T"],
            "wot": prep["WoIT"], "b0l": prep["b0L"], "bh0l": prep["bh0L"],
            "bh1l": prep["bh1L"], "zb": zb, "eye": eye,
            "w0r": prep["W0T"].reshape(1, D, H), "bml": bml,
        })

    res = run_bass_kernel_spmd(nc, in_maps, core_ids=list(range(NCORES)),
                               trace=trace)
    out = np.empty((B, D), dtype=np.float32)
    for c in range(NCORES):
        buf = res.results[c]["out"]                        # [128, 128]
        out[c * BC:(c + 1) * BC, :] = (
            buf.reshape(P, NJ, D).transpose(1, 0, 2).reshape(BC, D))
    return out, res


def kernel(**inputs):
    out, _ = _run(inputs, trace=False)
    return out


# revision 56
# speedup vs baseline: 1.2217x; 1.1813x over previous
"""MADE autoregressive sampler on 8 TRN2 NeuronCores — incremental frontier.

Strategy (vs. the full-recompute baseline):
- Data-parallel over batch: B=4096 -> 512 rows per core; weights replicated.
- Degree-sort hidden units. In MADE, a hidden unit's activation is FINAL once
  x columns 0..deg are set, so per AR step only the 1-2 "frontier" blocks
  (those containing degree idx-1) need recomputation. Everything else is
  computed once and cached:
    * z1 (layer-1 preact) kept in PSUM, updated by a rank-1 matmul per step.
    * S2/S3 = frozen off-diagonal partial sums per frontier block, cached in
      SBUF and restored into PSUM each step (then diag matmul accumulates).
    * theta (output-layer contributions of finalized blocks) accumulates in
      one PSUM bank, in batch-major chunk layout so the per-step tail ops are
      [128, 4] instead of [1, 512].
- fp16 operands everywhere (fp32 PSUM accumulation).
- Elementwise load spread across Scalar(Act)/Vector(DVE)/Pool(gpsimd).
"""

import os
import sys
import math
import hashlib
import numpy as np

for _p in ("/opt/trn_rl_repo", "/opt/pypackages"):
    if _p not in sys.path:
        sys.path.insert(0, _p)

import concourse.bass as bass
import concourse.tile as tile
from concourse import bacc
from concourse import mybir
from concourse.bass_utils import run_bass_kernel_spmd

D, H, L, B = 32, 1024, 2, 4096
NCORES = 8
BC = B // NCORES          # 512 batch rows per core
P = 128
NB = H // P               # 8 hidden blocks
NJ = BC // P              # 4 batch chunks of 128
F32 = mybir.dt.float32
F16 = mybir.dt.float16

DTYPE = os.environ.get("MADE_DTYPE", "fp16")
MMDT = {"fp16": mybir.dt.float16, "bf16": mybir.dt.bfloat16,
        "f32r": mybir.dt.float32r}[DTYPE]
NPDT = {"fp16": np.float16, "bf16": np.float32, "f32r": np.float32}[DTYPE]
STOP = int(os.environ.get("MADE_STOP", "32"))

AluOp = mybir.AluOpType
ActFn = mybir.ActivationFunctionType


def _schedule():
    """Static per-step schedule from the degree structure."""
    d_hid = np.arange(H) % (D - 1)
    perm = np.argsort(d_hid, kind="stable")
    ds = d_hid[perm]
    g_lo = [int(ds[P * b]) for b in range(NB)]
    g_hi = [int(ds[P * b + P - 1]) for b in range(NB)]
    entry = [g_lo[b] + 1 for b in range(NB)]
    final = [g_hi[b] + 1 for b in range(NB)]
    return perm, ds, g_lo, g_hi, entry, final


def _host_prep(W0, b0, Wh, bh, Wout, bout):
    d_in = np.arange(D)
    d_hid = np.arange(H) % (D - 1)
    d_out = np.arange(D) - 1
    m0 = (d_hid[:, None] >= d_in[None, :]).astype(np.float32)
    mh = (d_hid[:, None] >= d_hid[None, :]).astype(np.float32)
    mo = (d_out[:, None] >= d_hid[None, :]).astype(np.float32)
    mo = np.concatenate([mo, mo], axis=0)

    perm, ds, g_lo, g_hi, entry, final = _schedule()

    W0p = (m0 * W0)[perm]                     # [H, D]
    Wh0p = (mh * Wh[0])[perm][:, perm]        # [H, H] (out, in)
    Wh1p = (mh * Wh[1])[perm][:, perm]
    Wop = (mo * Wout)[:, perm]                # [2D, H]
    b0p = b0[perm]
    bh0p = bh[0][perm]
    bh1p = bh[1][perm]

    # lhsT layouts
    W0T = np.ascontiguousarray(W0p.T).astype(NPDT)          # [32, H]
    Wh0T = np.ascontiguousarray(
        Wh0p.T.reshape(NB, P, H).transpose(1, 0, 2)).astype(NPDT)  # [128, NB, H]
    Wh1T = np.ascontiguousarray(
        Wh1p.T.reshape(NB, P, H).transpose(1, 0, 2)).astype(NPDT)
    # interleaved output weights: col 2i = mu_i, col 2i+1 = ls_i
    WoI = np.empty((H, 2 * D), dtype=np.float32)
    WoI[:, 0::2] = Wop[:D, :].T
    WoI[:, 1::2] = Wop[D:, :].T
    WoIT = np.ascontiguousarray(
        WoI.reshape(NB, P, 2 * D).transpose(1, 0, 2)).astype(NPDT)  # [128, NB, 64]

    b0L = np.ascontiguousarray(b0p.reshape(NB, P).T).astype(np.float32)
    bh0L = np.ascontiguousarray(bh0p.reshape(NB, P).T).astype(np.float32)
    bh1L = np.ascontiguousarray(bh1p.reshape(NB, P).T).astype(np.float32)

    nzh0 = np.zeros((NB, NB), dtype=bool)
    nzh1 = np.zeros((NB, NB), dtype=bool)
    for r in range(NB):
        for c in range(NB):
            nzh0[r, c] = bool(np.any(Wh0p[r * P:(r + 1) * P, c * P:(c + 1) * P]))
            nzh1[r, c] = bool(np.any(Wh1p[r * P:(r + 1) * P, c * P:(c + 1) * P]))

    return dict(W0T=W0T, Wh0T=Wh0T, Wh1T=Wh1T, WoIT=WoIT,
                b0L=b0L, bh0L=bh0L, bh1L=bh1L,
                bout=bout.astype(np.float64),
                nzh0=nzh0, nzh1=nzh1,
                g_lo=g_lo, g_hi=g_hi, entry=entry, final=final)


def _build(prep):
    nc = bacc.Bacc("TRN2", target_bir_lowering=False, debug=False,
                   num_devices=NCORES)

    def din(name, shape, dt=F32):
        return nc.dram_tensor(name, list(shape), dt, kind="ExternalInput").ap()

    HB = BC // 2                                # 256: batch half per side
    d_w0 = din("w0t", (D, H), MMDT)
    d_w0r = din("w0r", (1, D, H), MMDT)
    d_wh0 = din("wh0t", (P, NB, H), MMDT)
    d_wh1 = din("wh1t", (P, NB, H), MMDT)
    d_wo = din("wot", (P, NB, 2 * D), MMDT)
    d_b0 = din("b0l", (P, NB))
    d_bh0 = din("bh0l", (P, NB))
    d_bh1 = din("bh1l", (P, NB))
    d_z = din("zb", (P, NJ * D), MMDT)          # batch-major [p, j*32+i]
    d_bml = din("bml", (P, NJ * 2 * D))         # bout replicated, interleaved
    d_eye = din("eye", (P, P), MMDT)
    d_out = nc.dram_tensor("out", [P, NJ * D], MMDT, kind="ExternalOutput").ap()

    bout = prep["bout"]
    nzh0, nzh1 = prep["nzh0"], prep["nzh1"]
    g_lo, g_hi = prep["g_lo"], prep["g_hi"]
    entry, final = prep["entry"], prep["final"]

    def active_at(idx):
        return [b for b in range(NB) if g_lo[b] <= idx - 1 <= g_hi[b]]

    from contextlib import ExitStack
    with tile.TileContext(nc) as tc, ExitStack() as ctx:
        cp = ctx.enter_context(tc.tile_pool(name="const", bufs=1))
        pp = ctx.enter_context(tc.tile_pool(name="psum", bufs=1, space="PSUM"))

        # ---- PSUM: exactly 8 banks (L = batch cols 0:256 -> Act side,
        #      R = cols 256:512 -> DVE side; separate tiles so the dep
        #      tracker lets Act/DVE halves run in parallel) ----
        pz1 = [pp.tile([P, HB], F32, tag=f"pz1{s}", name=f"pz1{s}") for s in "LR"]
        # one bank pair per block PARITY, shared by layers 2 and 3 (z2/z3
        # are chain-sequential), so the two straddle-step blocks overlap.
        # NOTE tags must not collide with the pz1 z1-bank tags.
        pzz = [[pp.tile([P, HB], F32, tag=f"pq{i}{s}", name=f"pq{i}{s}")
                for s in "LR"] for i in range(2)]
        pth = pp.tile([P, NJ, 2 * D], F32, tag="pth", name="pth")
        pmisc = pp.tile([P, 256, 2], F32, tag="pmisc", name="pmisc")
        # pmisc: [:, 0:4, :] pfr (j, mu/ls); partition0 cols 4:132 = xiT
        # chunks (4 x 128 fp16); [0:32, 132:164/164:196] xB-transpose ping/pong

        # ---- SBUF ----
        w0 = cp.tile([D, H], MMDT, tag="w0")
        w0r = cp.tile([1, D, H], MMDT, tag="w0r")
        wh0 = cp.tile([P, NB, H], MMDT, tag="wh0")
        wh1 = cp.tile([P, NB, H], MMDT, tag="wh1")
        wo = cp.tile([P, NB, 2 * D], MMDT, tag="wo")
        eye = cp.tile([P, P], MMDT, tag="eye")
        b0s = cp.tile([P, NB], F32, tag="b0s")
        bh0s = cp.tile([P, NB], F32, tag="bh0s")
        bh1s = cp.tile([P, NB], F32, tag="bh1s")
        zB = cp.tile([P, NJ, D], MMDT, tag="zB")
        xB = cp.tile([P, NJ, D], MMDT, tag="xB")
        xT4 = cp.tile([D, NJ, P], MMDT, tag="xT4")
        thetaS = cp.tile([P, NJ, 2 * D], F32, tag="thetaS")
        bml = cp.tile([P, NJ, 2 * D], F32, tag="bml")
        xiB = cp.tile([P, NJ], MMDT, tag="xiB")
        xiT = cp.tile([1, NJ, P], MMDT, tag="xiT")
        u8 = cp.tile([P, NJ, 2], MMDT, tag="u8")
        uls = cp.tile([P, NJ], MMDT, tag="uls")
        umu = cp.tile([P, NJ], MMDT, tag="umu")
        es = cp.tile([P, NJ], MMDT, tag="es")
        t2 = cp.tile([P, NJ], MMDT, tag="t2")
        aL = [[cp.tile([P, HB], MMDT, tag=f"a{l}L{r}", name=f"a{l}L{r}")
               for r in range(NB)] for l in range(3)]
        aR = [[cp.tile([P, HB], MMDT, tag=f"a{l}R{r}", name=f"a{l}R{r}")
               for r in range(NB)] for l in range(3)]
        # S caches, double-buffered by block parity
        S2p = [[cp.tile([P, HB], MMDT, tag=f"S2{s}{i}", name=f"S2{s}{i}")
                for s in "LR"] for i in range(2)]
        S3p = [[cp.tile([P, HB], MMDT, tag=f"S3{s}{i}", name=f"S3{s}{i}")
                for s in "LR"] for i in range(2)]
        z1nL = cp.tile([P, HB], MMDT, tag="z1nL")
        z1nR = cp.tile([P, HB], MMDT, tag="z1nR")

        # ---- DMA in; ~620ns each, serialized on one queue, so order by
        #      first use: everything step 0/1 touches goes first ----
        nc.sync.dma_start(zB[:], d_z)
        nc.sync.dma_start(eye[:], d_eye)
        nc.sync.dma_start(w0r[0:1, 0:8, :], d_w0r[0:1, 0:8, :])
        nc.sync.dma_start(b0s[:], d_b0)
        nc.sync.dma_start(wh0[:, 0, :], d_wh0[:, 0, :])
        nc.sync.dma_start(bh0s[:], d_bh0)
        nc.sync.dma_start(wh1[:, 0, :], d_wh1[:, 0, :])
        nc.sync.dma_start(bh1s[:], d_bh1)
        nc.sync.dma_start(wo[:], d_wo)
        nc.sync.dma_start(w0[:], d_w0)
        # theta/bias tensors are first read at step 4's finalize - defer them
        nc.sync.dma_start(thetaS[:], d_bml)      # theta starts as pure bias
        nc.sync.dma_start(bml[:], d_bml)
        for i in range(8, D, 8):
            nc.sync.dma_start(w0r[0:1, i:i + 8, :], d_w0r[0:1, i:i + 8, :])
        for c in range(1, NB):
            nc.sync.dma_start(wh0[:, c, :], d_wh0[:, c, :])
            nc.sync.dma_start(wh1[:, c, :], d_wh1[:, c, :])

        nc.vector.memset(xB[:], 0.0)

        xiTv = [pmisc[0:1, 4 + 32 * j:36 + 32 * j, :].bitcast(MMDT)
                for j in range(NJ)]
        xiTfull = pmisc[0:1, 4:132, :].bitcast(MMDT)         # [1,128,4] = 512
        xtt = [pmisc[0:D, 132:164, :].bitcast(MMDT),
               pmisc[0:D, 164:196, :].bitcast(MMDT)]

        def mm(out, lhsT, rhs, start, stop):
            nc.tensor.matmul(out, lhsT, rhs, start=start, stop=stop,
                             skip_group_check=True)

        def relu1(b):
            nc.scalar.activation(aL[0][b][:], pz1[0][:], ActFn.Relu,
                                 bias=b0s[:, b:b + 1], scale=1.0)
            nc.vector.tensor_scalar(aR[0][b][:], pz1[1][:],
                                    b0s[:, b:b + 1], 0.0, AluOp.add, AluOp.max)

        def relu2(b, pz):
            nc.scalar.activation(aL[1][b][:], pz[0][:], ActFn.Relu,
                                 bias=bh0s[:, b:b + 1], scale=1.0)
            nc.vector.tensor_scalar(aR[1][b][:], pz[1][:],
                                    bh0s[:, b:b + 1], 0.0, AluOp.add, AluOp.max)

        def relu3(b, pz):
            nc.scalar.activation(aL[2][b][:], pz[0][:], ActFn.Relu,
                                 bias=bh1s[:, b:b + 1], scale=1.0)
            nc.vector.tensor_scalar(aR[2][b][:], pz[1][:],
                                    bh1s[:, b:b + 1], 0.0, AluOp.add, AluOp.max)

        def xi_transpose():
            for j in range(NJ):
                nc.tensor.transpose(xiTv[j], xiB[:, j:j + 1], eye[:])
            nc.vector.tensor_scalar_add(xiT[:], xiTfull, 0.0)

        def layer_mms(pz, wh, a_in, b, cols, use_S, Ssb):
            """Accumulate one hidden layer for block b into pz (L and R)."""
            if use_S:
                todo = [c for c in cols if c >= b]
                for side in range(2):
                    mm(pz[side][:], eye[:], Ssb[side][:], True, False)
                    for k, c in enumerate(todo):
                        mm(pz[side][:], wh[:, c, b * P:(b + 1) * P],
                           a_in[side][c][:], False, k == len(todo) - 1)
            else:
                for side in range(2):
                    for k, c in enumerate(cols):
                        mm(pz[side][:], wh[:, c, b * P:(b + 1) * P],
                           a_in[side][c][:], k == 0, k == len(cols) - 1)

        # ---- step 0: x_0 = z_0 * exp(bout[D]) + bout[0] ----
        s0 = float(math.exp(bout[D]))
        m0c = float(bout[0])
        nc.vector.tensor_scalar(xiB[:], zB[:, :, 0], s0, m0c,
                                AluOp.mult, AluOp.add)
        nc.gpsimd.tensor_scalar_add(xB[:, :, 0], xiB[:], 0.0)
        xi_transpose()

        S2ready = [False] * NB
        theta_init = [False] * NJ

        for idx in range(1, STOP):
            act_blocks = active_at(idx)
            b_old = act_blocks[0]
            b_new = act_blocks[1] if len(act_blocks) > 1 else None
            ent = [b for b in act_blocks if entry[b] == idx]
            finalizing = [b for b in act_blocks if final[b] == idx]

            # rank-1 z1 for the persisted block (not for entering block;
            # block 0 "enters" at step 1 with a plain start=True rank-1)
            first = (b_old == 0 and idx == 1)
            if entry[b_old] != idx or first:
                mm(pz1[0][:], w0r[0:1, idx - 1, b_old * P:(b_old + 1) * P],
                   xiT[0:1, 0:NJ // 2, :], first, True)
                mm(pz1[1][:], w0r[0:1, idx - 1, b_old * P:(b_old + 1) * P],
                   xiT[0:1, NJ // 2:NJ, :], first, True)
            relu1(b_old)

            # entering block: overwrite pz1 after old relu1 read
            if b_new is not None:
                mm(pz1[0][:], eye[:], z1nL[:], True, False)
                mm(pz1[0][:], w0r[0:1, idx - 1, b_new * P:(b_new + 1) * P],
                   xiT[0:1, 0:NJ // 2, :], False, True)
                mm(pz1[1][:], eye[:], z1nR[:], True, False)
                mm(pz1[1][:], w0r[0:1, idx - 1, b_new * P:(b_new + 1) * P],
                   xiT[0:1, NJ // 2:NJ, :], False, True)
                relu1(b_new)

            # -- layer 2 --
            a1 = (aL[0], aR[0])
            a2 = (aL[1], aR[1])
            a3 = (aL[2], aR[2])
            def enter_layer(pz, wh, a_in, b, cols, Sp):
                """Entering block's layer: restore prefetched S' (c<=b-2),
                then the c=b-1 term and the diagonal. Correct because
                a1/a2[b-1] are FINAL this step."""
                for side in range(2):
                    got = False
                    if any(c <= b - 2 for c in cols):
                        mm(pz[side][:], eye[:], Sp[side][:], True, False)
                        got = True
                    for c in cols:
                        if c == b - 1:
                            mm(pz[side][:], wh[:, c, b * P:(b + 1) * P],
                               a_in[side][c][:], not got, False)
                            got = True
                    mm(pz[side][:], wh[:, b, b * P:(b + 1) * P],
                       a_in[side][b][:], False, True)

            # bank assignment: steady steps alternate z2/z3 across the two
            # pairs (restores run early, off-chain); straddle steps give each
            # block its own dedicated pair so the two pipelines overlap.
            z2o = pzz[b_old % 2]
            z3o = pzz[b_old % 2] if b_new is not None else pzz[(b_old + 1) % 2]
            cols2_old = [c for c in range(NB)
                         if nzh0[b_old, c] and g_lo[c] <= idx - 1]
            layer_mms(z2o, wh0, a1, b_old, cols2_old,
                      S2ready[b_old], S2p[b_old % 2])
            relu2(b_old, z2o)
            if b_new is not None:
                cols2_new = [c for c in range(NB)
                             if nzh0[b_new, c] and g_lo[c] <= idx - 1]
                enter_layer(pzz[b_new % 2], wh0, a1, b_new, cols2_new,
                            S2p[b_new % 2])
                relu2(b_new, pzz[b_new % 2])

            # -- layer 3 (old block may need a2[b_new]: emitted after) --
            cols3_old = [c for c in range(NB)
                         if nzh1[b_old, c] and g_lo[c] <= idx - 1]
            layer_mms(z3o, wh1, a2, b_old, cols3_old,
                      S2ready[b_old], S3p[b_old % 2])
            relu3(b_old, z3o)
            if b_new is not None:
                cols3_new = [c for c in range(NB)
                             if nzh1[b_new, c] and g_lo[c] <= idx - 1]
                enter_layer(pzz[b_new % 2], wh1, a2, b_new, cols3_new,
                            S3p[b_new % 2])
                relu3(b_new, pzz[b_new % 2])

            # -- frontier output contribution (batch-major, N=2); theta (with
            #    folded biases) is accumulated INTO the same psum by early
            #    identity-matmuls, so exp reads psum directly and the tail's
            #    DVE add disappears --
            for j in range(NJ):
                side, jj = (0, j) if j < NJ // 2 else (1, j - NJ // 2)
                for k, b in enumerate(act_blocks):
                    mm(pmisc[:, j, 0:2],
                       a3[side][b][:, jj * P:(jj + 1) * P],
                       wo[:, b, 2 * idx:2 * idx + 2],
                       k == 0, k == len(act_blocks) - 1)

            # -- tail (before the first finalize, theta is pure bias) --
            if idx <= final[0]:
                nc.vector.tensor_scalar_add(uls[:], pmisc[:, 0:NJ, 1],
                                            float(bout[idx + D]))
                nc.vector.tensor_scalar_add(umu[:], pmisc[:, 0:NJ, 0],
                                            float(bout[idx]))
                nc.scalar.activation(es[:], uls[:], ActFn.Exp)
                nc.gpsimd.tensor_tensor(t2[:], es[:], zB[:, :, idx],
                                        AluOp.mult)
                nc.gpsimd.tensor_tensor(xiB[:], t2[:], umu[:], AluOp.add)
            else:
                nc.vector.tensor_tensor(u8[:], pmisc[:, 0:NJ, :],
                                        thetaS[:, :, 2 * idx:2 * idx + 2],
                                        AluOp.add)
                nc.scalar.activation(es[:], u8[:, :, 1], ActFn.Exp)
                nc.gpsimd.tensor_tensor(t2[:], es[:], zB[:, :, idx],
                                        AluOp.mult)
                nc.gpsimd.tensor_tensor(xiB[:], t2[:], u8[:, :, 0],
                                        AluOp.add)
            if idx < STOP - 1:
                xi_transpose()
            nc.gpsimd.tensor_scalar_add(xB[:, :, idx], xiB[:], 0.0)

            # -- finalize theta (after tail read of pmisc/thetaS) --
            for b in finalizing:
                if idx >= STOP - 1:
                    continue
                for j in range(NJ):
                    side, jj = (0, j) if j < NJ // 2 else (1, j - NJ // 2)
                    mm(pth[:, j, :],
                       a3[side][b][:, jj * P:(jj + 1) * P],
                       wo[:, b, :],
                       not theta_init[j], True)
                    theta_init[j] = True
                nc.vector.tensor_tensor(thetaS[:], pth[:], bml[:], AluOp.add)

            # -- full S2/S3 cache for the lone active block, one step after
            #    entry; runs in the free bank pairs, hidden in chain slack --
            b = b_old
            if b > 0 and not S2ready[b] and idx == entry[b] + 1 \
                    and idx < final[b]:
                cc2 = [c for c in range(NB) if nzh0[b, c] and c < b]
                cc3 = [c for c in range(NB) if nzh1[b, c] and c < b]
                scr2 = pzz[b % 2]          # free after this step's relu2
                scr3 = pzz[(b + 1) % 2]    # free after this step's relu3
                for side in range(2):
                    for k, c in enumerate(cc2):
                        mm(scr2[side][:], wh0[:, c, b * P:(b + 1) * P],
                           a1[side][c][:], k == 0, k == len(cc2) - 1)
                nc.scalar.copy(S2p[b % 2][0][:], scr2[0][:])
                nc.vector.tensor_scalar_add(S2p[b % 2][1][:], scr2[1][:], 0.0)
                for side in range(2):
                    for k, c in enumerate(cc3):
                        mm(scr3[side][:], wh1[:, c, b * P:(b + 1) * P],
                           a2[side][c][:], k == 0, k == len(cc3) - 1)
                nc.scalar.copy(S3p[b % 2][0][:], scr3[0][:])
                nc.vector.tensor_scalar_add(S3p[b % 2][1][:], scr3[1][:], 0.0)
                S2ready[b] = True
            if final[b_old] == idx:
                S2ready[b_old] = False

            # -- prefetch for the block entering next step: z1 and the
            #    S' partial sums over already-final inputs (c <= bb-2) --
            pre = [bb for bb in range(1, NB) if entry[bb] == idx + 1]
            for bb in pre:
                for j in range(NJ):
                    pg = xtt[j % 2]
                    nc.tensor.transpose(pg, xB[:, j, :], eye[:])
                    nc.vector.tensor_scalar_add(xT4[:, j, :], pg, 0.0)
                scr = None
                scr = pzz[bb % 2]
                mm(scr[0][:], w0[:, bb * P:(bb + 1) * P],
                   xT4[:, 0:NJ // 2, :], True, True)
                mm(scr[1][:], w0[:, bb * P:(bb + 1) * P],
                   xT4[:, NJ // 2:NJ, :], True, True)
                nc.scalar.copy(z1nL[:], scr[0][:])
                nc.vector.tensor_scalar_add(z1nR[:], scr[1][:], 0.0)
                cc2 = [c for c in range(NB) if nzh0[bb, c] and c <= bb - 2]
                cc3 = [c for c in range(NB) if nzh1[bb, c] and c <= bb - 2]
                if cc2:
                    scr2 = pzz[(bb + 1) % 2]
                    for side in range(2):
                        for k, c in enumerate(cc2):
                            mm(scr2[side][:], wh0[:, c, bb * P:(bb + 1) * P],
                               a1[side][c][:], k == 0, k == len(cc2) - 1)
                    nc.scalar.copy(S2p[bb % 2][0][:], scr2[0][:])
                    nc.vector.tensor_scalar_add(S2p[bb % 2][1][:],
                                                scr2[1][:], 0.0)
                if cc3:
                    for side in range(2):
                        for k, c in enumerate(cc3):
                            mm(scr[side][:], wh1[:, c, bb * P:(bb + 1) * P],
                               a2[side][c][:], k == 0, k == len(cc3) - 1)
                    nc.scalar.copy(S3p[bb % 2][0][:], scr[0][:])
                    nc.vector.tensor_scalar_add(S3p[bb % 2][1][:],
                                                scr[1][:], 0.0)

        # ---- output (fp16; host converts) ----
        nc.sync.dma_start(d_out, xB[:])

    nc.compile()
    return nc


_CACHE = {}


def _get_program(prep):
    key = (DTYPE, STOP, hashlib.md5(prep["bout"].tobytes()).hexdigest())
    if key not in _CACHE:
        _CACHE[key] = _build(prep)
    return _CACHE[key]


def _run(inputs, trace=False):
    z = np.asarray(inputs["z"], dtype=np.float32)
    prep = _host_prep(np.asarray(inputs["W0"], np.float32),
                      np.asarray(inputs["b0"], np.float32),
                      np.asarray(inputs["Wh"], np.float32),
                      np.asarray(inputs["bh"], np.float32),
                      np.asarray(inputs["Wout"], np.float32),
                      np.asarray(inputs["bout"], np.float32))
    nc = _get_program(prep)

    eye = np.eye(P, dtype=NPDT)
    bout32 = prep["bout"].astype(np.float32)
    bml1 = np.empty(2 * D, dtype=np.float32)
    bml1[0::2] = bout32[:D]
    bml1[1::2] = bout32[D:]
    bml = np.ascontiguousarray(
        np.broadcast_to(np.tile(bml1, NJ), (P, NJ * 2 * D))).astype(np.float32)
    in_maps = []
    for c in range(NCORES):
        zs = z[c * BC:(c + 1) * BC, :]                     # [512, 32]
        # batch-major: [p, j, i] = z[j*128+p, i]
        zb = np.ascontiguousarray(
            zs.reshape(NJ, P, D).transpose(1, 0, 2).reshape(P, NJ * D)
        ).astype(NPDT)
        in_maps.append({
            "w0t": prep["W0T"], "wh0t": prep["Wh0T"], "wh1t": prep["Wh1T"],
            "wot": prep["WoIT"], "b0l": prep["b0L"], "bh0l": prep["bh0L"],
            "bh1l": prep["bh1L"], "zb": zb, "eye": eye,
            "w0r": prep["W0T"].reshape(1, D, H), "bml": bml,
        })

    res = run_bass_kernel_spmd(nc, in_maps, core_ids=list(range(NCORES)),
                               trace=trace)
    out = np.empty((B, D), dtype=np.float32)
    for c in range(NCORES):
        buf = res.results[c]["out"].astype(np.float32)     # [128, 128]
        out[c * BC:(c + 1) * BC, :] = (
            buf.reshape(P, NJ, D).transpose(1, 0, 2).reshape(BC, D))
    return out, res


def kernel(**inputs):
    out, _ = _run(inputs, trace=False)
    return out


# revision 57
# speedup vs baseline: 1.2416x; 1.0162x over previous
"""MADE autoregressive sampler on 8 TRN2 NeuronCores — incremental frontier.

Strategy (vs. the full-recompute baseline):
- Data-parallel over batch: B=4096 -> 512 rows per core; weights replicated.
- Degree-sort hidden units. In MADE, a hidden unit's activation is FINAL once
  x columns 0..deg are set, so per AR step only the 1-2 "frontier" blocks
  (those containing degree idx-1) need recomputation. Everything else is
  computed once and cached:
    * z1 (layer-1 preact) kept in PSUM, updated by a rank-1 matmul per step.
    * S2/S3 = frozen off-diagonal partial sums per frontier block, cached in
      SBUF and restored into PSUM each step (then diag matmul accumulates).
    * theta (output-layer contributions of finalized blocks) accumulates in
      one PSUM bank, in batch-major chunk layout so the per-step tail ops are
      [128, 4] instead of [1, 512].
- fp16 operands everywhere (fp32 PSUM accumulation).
- Elementwise load spread across Scalar(Act)/Vector(DVE)/Pool(gpsimd).
"""

import os
import sys
import math
import hashlib
import numpy as np

for _p in ("/opt/trn_rl_repo", "/opt/pypackages"):
    if _p not in sys.path:
        sys.path.insert(0, _p)

import concourse.bass as bass
import concourse.tile as tile
from concourse import bacc
from concourse import mybir
from concourse.bass_utils import run_bass_kernel_spmd

D, H, L, B = 32, 1024, 2, 4096
NCORES = 8
BC = B // NCORES          # 512 batch rows per core
P = 128
NB = H // P               # 8 hidden blocks
NJ = BC // P              # 4 batch chunks of 128
F32 = mybir.dt.float32
F16 = mybir.dt.float16

DTYPE = os.environ.get("MADE_DTYPE", "fp16")
MMDT = {"fp16": mybir.dt.float16, "bf16": mybir.dt.bfloat16,
        "f32r": mybir.dt.float32r}[DTYPE]
NPDT = {"fp16": np.float16, "bf16": np.float32, "f32r": np.float32}[DTYPE]
STOP = int(os.environ.get("MADE_STOP", "32"))

AluOp = mybir.AluOpType
ActFn = mybir.ActivationFunctionType


def _schedule():
    """Static per-step schedule from the degree structure."""
    d_hid = np.arange(H) % (D - 1)
    perm = np.argsort(d_hid, kind="stable")
    ds = d_hid[perm]
    g_lo = [int(ds[P * b]) for b in range(NB)]
    g_hi = [int(ds[P * b + P - 1]) for b in range(NB)]
    entry = [g_lo[b] + 1 for b in range(NB)]
    final = [g_hi[b] + 1 for b in range(NB)]
    return perm, ds, g_lo, g_hi, entry, final


def _host_prep(W0, b0, Wh, bh, Wout, bout):
    d_in = np.arange(D)
    d_hid = np.arange(H) % (D - 1)
    d_out = np.arange(D) - 1
    m0 = (d_hid[:, None] >= d_in[None, :]).astype(np.float32)
    mh = (d_hid[:, None] >= d_hid[None, :]).astype(np.float32)
    mo = (d_out[:, None] >= d_hid[None, :]).astype(np.float32)
    mo = np.concatenate([mo, mo], axis=0)

    perm, ds, g_lo, g_hi, entry, final = _schedule()

    W0p = (m0 * W0)[perm]                     # [H, D]
    Wh0p = (mh * Wh[0])[perm][:, perm]        # [H, H] (out, in)
    Wh1p = (mh * Wh[1])[perm][:, perm]
    Wop = (mo * Wout)[:, perm]                # [2D, H]
    b0p = b0[perm]
    bh0p = bh[0][perm]
    bh1p = bh[1][perm]

    # lhsT layouts
    W0T = np.ascontiguousarray(W0p.T).astype(NPDT)          # [32, H]
    Wh0T = np.ascontiguousarray(
        Wh0p.T.reshape(NB, P, H).transpose(1, 0, 2)).astype(NPDT)  # [128, NB, H]
    Wh1T = np.ascontiguousarray(
        Wh1p.T.reshape(NB, P, H).transpose(1, 0, 2)).astype(NPDT)
    # interleaved output weights: col 2i = mu_i, col 2i+1 = ls_i
    WoI = np.empty((H, 2 * D), dtype=np.float32)
    WoI[:, 0::2] = Wop[:D, :].T
    WoI[:, 1::2] = Wop[D:, :].T
    WoIT = np.ascontiguousarray(
        WoI.reshape(NB, P, 2 * D).transpose(1, 0, 2)).astype(NPDT)  # [128, NB, 64]

    b0L = np.ascontiguousarray(b0p.reshape(NB, P).T).astype(np.float32)
    bh0L = np.ascontiguousarray(bh0p.reshape(NB, P).T).astype(np.float32)
    bh1L = np.ascontiguousarray(bh1p.reshape(NB, P).T).astype(np.float32)

    nzh0 = np.zeros((NB, NB), dtype=bool)
    nzh1 = np.zeros((NB, NB), dtype=bool)
    for r in range(NB):
        for c in range(NB):
            nzh0[r, c] = bool(np.any(Wh0p[r * P:(r + 1) * P, c * P:(c + 1) * P]))
            nzh1[r, c] = bool(np.any(Wh1p[r * P:(r + 1) * P, c * P:(c + 1) * P]))

    return dict(W0T=W0T, Wh0T=Wh0T, Wh1T=Wh1T, WoIT=WoIT,
                b0L=b0L, bh0L=bh0L, bh1L=bh1L,
                bout=bout.astype(np.float64),
                nzh0=nzh0, nzh1=nzh1,
                g_lo=g_lo, g_hi=g_hi, entry=entry, final=final)


def _build(prep):
    nc = bacc.Bacc("TRN2", target_bir_lowering=False, debug=False,
                   num_devices=NCORES)

    def din(name, shape, dt=F32):
        return nc.dram_tensor(name, list(shape), dt, kind="ExternalInput").ap()

    HB = BC // 2                                # 256: batch half per side
    d_w0 = din("w0t", (D, H), MMDT)
    d_w0r = din("w0r", (1, D, H), MMDT)
    d_wh0 = din("wh0t", (P, NB, H), MMDT)
    d_wh1 = din("wh1t", (P, NB, H), MMDT)
    d_wo = din("wot", (P, NB, 2 * D), MMDT)
    d_b0 = din("b0l", (P, NB))
    d_bh0 = din("bh0l", (P, NB))
    d_bh1 = din("bh1l", (P, NB))
    d_z = din("zb", (P, NJ * D), MMDT)          # batch-major [p, j*32+i]
    d_bml = din("bml", (P, NJ * 2 * D))         # bout replicated, interleaved
    d_eye = din("eye", (P, P), MMDT)
    d_out = nc.dram_tensor("out", [P, NJ * D], F32, kind="ExternalOutput").ap()

    bout = prep["bout"]
    nzh0, nzh1 = prep["nzh0"], prep["nzh1"]
    g_lo, g_hi = prep["g_lo"], prep["g_hi"]
    entry, final = prep["entry"], prep["final"]

    def active_at(idx):
        return [b for b in range(NB) if g_lo[b] <= idx - 1 <= g_hi[b]]

    from contextlib import ExitStack
    with tile.TileContext(nc) as tc, ExitStack() as ctx:
        cp = ctx.enter_context(tc.tile_pool(name="const", bufs=1))
        pp = ctx.enter_context(tc.tile_pool(name="psum", bufs=1, space="PSUM"))

        # ---- PSUM: exactly 8 banks (L = batch cols 0:256 -> Act side,
        #      R = cols 256:512 -> DVE side; separate tiles so the dep
        #      tracker lets Act/DVE halves run in parallel) ----
        pz1 = [pp.tile([P, HB], F32, tag=f"pz1{s}", name=f"pz1{s}") for s in "LR"]
        # one bank pair per block PARITY, shared by layers 2 and 3 (z2/z3
        # are chain-sequential), so the two straddle-step blocks overlap.
        # NOTE tags must not collide with the pz1 z1-bank tags.
        pzz = [[pp.tile([P, HB], F32, tag=f"pq{i}{s}", name=f"pq{i}{s}")
                for s in "LR"] for i in range(2)]
        pth = pp.tile([P, NJ, 2 * D], F32, tag="pth", name="pth")
        pmisc = pp.tile([P, 256, 2], F32, tag="pmisc", name="pmisc")
        # pmisc: [:, 0:4, :] pfr (j, mu/ls); partition0 cols 4:132 = xiT
        # chunks (4 x 128 fp16); [0:32, 132:164/164:196] xB-transpose ping/pong

        # ---- SBUF ----
        w0 = cp.tile([D, H], MMDT, tag="w0")
        w0r = cp.tile([1, D, H], MMDT, tag="w0r")
        wh0 = cp.tile([P, NB, H], MMDT, tag="wh0")
        wh1 = cp.tile([P, NB, H], MMDT, tag="wh1")
        wo = cp.tile([P, NB, 2 * D], MMDT, tag="wo")
        eye = cp.tile([P, P], MMDT, tag="eye")
        b0s = cp.tile([P, NB], F32, tag="b0s")
        bh0s = cp.tile([P, NB], F32, tag="bh0s")
        bh1s = cp.tile([P, NB], F32, tag="bh1s")
        zB = cp.tile([P, NJ, D], MMDT, tag="zB")
        xB = cp.tile([P, NJ, D], MMDT, tag="xB")
        xBf = cp.tile([P, NJ * D], F32, tag="xBf")
        xT4 = cp.tile([D, NJ, P], MMDT, tag="xT4")
        thetaS = cp.tile([P, NJ, 2 * D], F32, tag="thetaS")
        bml = cp.tile([P, NJ, 2 * D], F32, tag="bml")
        xiB = cp.tile([P, NJ], MMDT, tag="xiB")
        xiT = cp.tile([1, NJ, P], MMDT, tag="xiT")
        u8 = cp.tile([P, NJ, 2], MMDT, tag="u8")
        es = cp.tile([P, NJ], MMDT, tag="es")
        t2 = cp.tile([P, NJ], MMDT, tag="t2")
        aL = [[cp.tile([P, HB], MMDT, tag=f"a{l}L{r}", name=f"a{l}L{r}")
               for r in range(NB)] for l in range(3)]
        aR = [[cp.tile([P, HB], MMDT, tag=f"a{l}R{r}", name=f"a{l}R{r}")
               for r in range(NB)] for l in range(3)]
        # S caches, double-buffered by block parity
        S2p = [[cp.tile([P, HB], MMDT, tag=f"S2{s}{i}", name=f"S2{s}{i}")
                for s in "LR"] for i in range(2)]
        S3p = [[cp.tile([P, HB], MMDT, tag=f"S3{s}{i}", name=f"S3{s}{i}")
                for s in "LR"] for i in range(2)]
        z1nL = cp.tile([P, HB], MMDT, tag="z1nL")
        z1nR = cp.tile([P, HB], MMDT, tag="z1nR")

        # ---- DMA in; ~620ns each, serialized on one queue, so order by
        #      first use: everything step 0/1 touches goes first ----
        nc.sync.dma_start(zB[:], d_z)
        nc.sync.dma_start(eye[:], d_eye)
        nc.sync.dma_start(w0r[0:1, 0:8, :], d_w0r[0:1, 0:8, :])
        nc.sync.dma_start(b0s[:], d_b0)
        nc.sync.dma_start(wh0[:, 0, :], d_wh0[:, 0, :])
        nc.sync.dma_start(bh0s[:], d_bh0)
        nc.sync.dma_start(wh1[:, 0, :], d_wh1[:, 0, :])
        nc.sync.dma_start(bh1s[:], d_bh1)
        nc.sync.dma_start(wo[:], d_wo)
        nc.sync.dma_start(thetaS[:], d_bml)      # theta starts as pure bias
        nc.sync.dma_start(bml[:], d_bml)
        nc.sync.dma_start(w0[:], d_w0)
        for i in range(8, D, 8):
            nc.sync.dma_start(w0r[0:1, i:i + 8, :], d_w0r[0:1, i:i + 8, :])
        for c in range(1, NB):
            nc.sync.dma_start(wh0[:, c, :], d_wh0[:, c, :])
            nc.sync.dma_start(wh1[:, c, :], d_wh1[:, c, :])

        nc.vector.memset(xB[:], 0.0)

        xiTv = [pmisc[0:1, 4 + 32 * j:36 + 32 * j, :].bitcast(MMDT)
                for j in range(NJ)]
        xiTfull = pmisc[0:1, 4:132, :].bitcast(MMDT)         # [1,128,4] = 512
        xtt = [pmisc[0:D, 132:164, :].bitcast(MMDT),
               pmisc[0:D, 164:196, :].bitcast(MMDT)]

        def mm(out, lhsT, rhs, start, stop):
            nc.tensor.matmul(out, lhsT, rhs, start=start, stop=stop,
                             skip_group_check=True)

        def relu1(b):
            nc.scalar.activation(aL[0][b][:], pz1[0][:], ActFn.Relu,
                                 bias=b0s[:, b:b + 1], scale=1.0)
            nc.vector.tensor_scalar(aR[0][b][:], pz1[1][:],
                                    b0s[:, b:b + 1], 0.0, AluOp.add, AluOp.max)

        def relu2(b, pz):
            nc.scalar.activation(aL[1][b][:], pz[0][:], ActFn.Relu,
                                 bias=bh0s[:, b:b + 1], scale=1.0)
            nc.vector.tensor_scalar(aR[1][b][:], pz[1][:],
                                    bh0s[:, b:b + 1], 0.0, AluOp.add, AluOp.max)

        def relu3(b, pz):
            nc.scalar.activation(aL[2][b][:], pz[0][:], ActFn.Relu,
                                 bias=bh1s[:, b:b + 1], scale=1.0)
            nc.vector.tensor_scalar(aR[2][b][:], pz[1][:],
                                    bh1s[:, b:b + 1], 0.0, AluOp.add, AluOp.max)

        def xi_transpose():
            for j in range(NJ):
                nc.tensor.transpose(xiTv[j], xiB[:, j:j + 1], eye[:])
            nc.vector.tensor_scalar_add(xiT[:], xiTfull, 0.0)

        def layer_mms(pz, wh, a_in, b, cols, use_S, Ssb):
            """Accumulate one hidden layer for block b into pz (L and R)."""
            if use_S:
                todo = [c for c in cols if c >= b]
                for side in range(2):
                    mm(pz[side][:], eye[:], Ssb[side][:], True, False)
                    for k, c in enumerate(todo):
                        mm(pz[side][:], wh[:, c, b * P:(b + 1) * P],
                           a_in[side][c][:], False, k == len(todo) - 1)
            else:
                for side in range(2):
                    for k, c in enumerate(cols):
                        mm(pz[side][:], wh[:, c, b * P:(b + 1) * P],
                           a_in[side][c][:], k == 0, k == len(cols) - 1)

        # ---- step 0: x_0 = z_0 * exp(bout[D]) + bout[0] ----
        s0 = float(math.exp(bout[D]))
        m0c = float(bout[0])
        nc.vector.tensor_scalar(xiB[:], zB[:, :, 0], s0, m0c,
                                AluOp.mult, AluOp.add)
        nc.gpsimd.tensor_scalar_add(xB[:, :, 0], xiB[:], 0.0)
        xi_transpose()

        S2ready = [False] * NB
        theta_init = [False] * NJ

        for idx in range(1, STOP):
            act_blocks = active_at(idx)
            b_old = act_blocks[0]
            b_new = act_blocks[1] if len(act_blocks) > 1 else None
            ent = [b for b in act_blocks if entry[b] == idx]
            finalizing = [b for b in act_blocks if final[b] == idx]

            # rank-1 z1 for the persisted block (not for entering block;
            # block 0 "enters" at step 1 with a plain start=True rank-1)
            first = (b_old == 0 and idx == 1)
            if entry[b_old] != idx or first:
                mm(pz1[0][:], w0r[0:1, idx - 1, b_old * P:(b_old + 1) * P],
                   xiT[0:1, 0:NJ // 2, :], first, True)
                mm(pz1[1][:], w0r[0:1, idx - 1, b_old * P:(b_old + 1) * P],
                   xiT[0:1, NJ // 2:NJ, :], first, True)
            relu1(b_old)

            # entering block: overwrite pz1 after old relu1 read
            if b_new is not None:
                mm(pz1[0][:], eye[:], z1nL[:], True, False)
                mm(pz1[0][:], w0r[0:1, idx - 1, b_new * P:(b_new + 1) * P],
                   xiT[0:1, 0:NJ // 2, :], False, True)
                mm(pz1[1][:], eye[:], z1nR[:], True, False)
                mm(pz1[1][:], w0r[0:1, idx - 1, b_new * P:(b_new + 1) * P],
                   xiT[0:1, NJ // 2:NJ, :], False, True)
                relu1(b_new)

            # -- layer 2 --
            a1 = (aL[0], aR[0])
            a2 = (aL[1], aR[1])
            a3 = (aL[2], aR[2])
            def enter_layer(pz, wh, a_in, b, cols, Sp):
                """Entering block's layer: restore prefetched S' (c<=b-2),
                then the c=b-1 term and the diagonal. Correct because
                a1/a2[b-1] are FINAL this step."""
                for side in range(2):
                    got = False
                    if any(c <= b - 2 for c in cols):
                        mm(pz[side][:], eye[:], Sp[side][:], True, False)
                        got = True
                    for c in cols:
                        if c == b - 1:
                            mm(pz[side][:], wh[:, c, b * P:(b + 1) * P],
                               a_in[side][c][:], not got, False)
                            got = True
                    mm(pz[side][:], wh[:, b, b * P:(b + 1) * P],
                       a_in[side][b][:], False, True)

            # bank assignment: steady steps alternate z2/z3 across the two
            # pairs (restores run early, off-chain); straddle steps give each
            # block its own dedicated pair so the two pipelines overlap.
            z2o = pzz[b_old % 2]
            z3o = pzz[b_old % 2] if b_new is not None else pzz[(b_old + 1) % 2]
            cols2_old = [c for c in range(NB)
                         if nzh0[b_old, c] and g_lo[c] <= idx - 1]
            layer_mms(z2o, wh0, a1, b_old, cols2_old,
                      S2ready[b_old], S2p[b_old % 2])
            relu2(b_old, z2o)
            if b_new is not None:
                cols2_new = [c for c in range(NB)
                             if nzh0[b_new, c] and g_lo[c] <= idx - 1]
                enter_layer(pzz[b_new % 2], wh0, a1, b_new, cols2_new,
                            S2p[b_new % 2])
                relu2(b_new, pzz[b_new % 2])

            # -- layer 3 (old block may need a2[b_new]: emitted after) --
            cols3_old = [c for c in range(NB)
                         if nzh1[b_old, c] and g_lo[c] <= idx - 1]
            layer_mms(z3o, wh1, a2, b_old, cols3_old,
                      S2ready[b_old], S3p[b_old % 2])
            relu3(b_old, z3o)
            if b_new is not None:
                cols3_new = [c for c in range(NB)
                             if nzh1[b_new, c] and g_lo[c] <= idx - 1]
                enter_layer(pzz[b_new % 2], wh1, a2, b_new, cols3_new,
                            S3p[b_new % 2])
                relu3(b_new, pzz[b_new % 2])

            # -- frontier output contribution (batch-major, N=2); theta (with
            #    folded biases) is accumulated INTO the same psum by early
            #    identity-matmuls, so exp reads psum directly and the tail's
            #    DVE add disappears --
            for j in range(NJ):
                side, jj = (0, j) if j < NJ // 2 else (1, j - NJ // 2)
                for k, b in enumerate(act_blocks):
                    mm(pmisc[:, j, 0:2],
                       a3[side][b][:, jj * P:(jj + 1) * P],
                       wo[:, b, 2 * idx:2 * idx + 2],
                       k == 0, k == len(act_blocks) - 1)

            # -- tail --
            nc.vector.tensor_tensor(u8[:], pmisc[:, 0:NJ, :],
                                    thetaS[:, :, 2 * idx:2 * idx + 2],
                                    AluOp.add)
            nc.scalar.activation(es[:], u8[:, :, 1], ActFn.Exp)
            nc.gpsimd.tensor_tensor(t2[:], es[:], zB[:, :, idx], AluOp.mult)
            nc.gpsimd.tensor_tensor(xiB[:], t2[:], u8[:, :, 0], AluOp.add)
            if idx < STOP - 1:
                xi_transpose()
            nc.gpsimd.tensor_scalar_add(xB[:, :, idx], xiB[:], 0.0)

            # -- finalize theta (after tail read of pmisc/thetaS) --
            for b in finalizing:
                if idx >= STOP - 1:
                    continue
                for j in range(NJ):
                    side, jj = (0, j) if j < NJ // 2 else (1, j - NJ // 2)
                    mm(pth[:, j, :],
                       a3[side][b][:, jj * P:(jj + 1) * P],
                       wo[:, b, :],
                       not theta_init[j], True)
                    theta_init[j] = True
                nc.vector.tensor_tensor(thetaS[:], pth[:], bml[:], AluOp.add)

            # -- full S2/S3 cache for the lone active block, one step after
            #    entry; runs in the free bank pairs, hidden in chain slack --
            b = b_old
            if b > 0 and not S2ready[b] and idx == entry[b] + 1 \
                    and idx < final[b]:
                cc2 = [c for c in range(NB) if nzh0[b, c] and c < b]
                cc3 = [c for c in range(NB) if nzh1[b, c] and c < b]
                scr2 = pzz[b % 2]          # free after this step's relu2
                scr3 = pzz[(b + 1) % 2]    # free after this step's relu3
                for side in range(2):
                    for k, c in enumerate(cc2):
                        mm(scr2[side][:], wh0[:, c, b * P:(b + 1) * P],
                           a1[side][c][:], k == 0, k == len(cc2) - 1)
                nc.scalar.copy(S2p[b % 2][0][:], scr2[0][:])
                nc.vector.tensor_scalar_add(S2p[b % 2][1][:], scr2[1][:], 0.0)
                for side in range(2):
                    for k, c in enumerate(cc3):
                        mm(scr3[side][:], wh1[:, c, b * P:(b + 1) * P],
                           a2[side][c][:], k == 0, k == len(cc3) - 1)
                nc.scalar.copy(S3p[b % 2][0][:], scr3[0][:])
                nc.vector.tensor_scalar_add(S3p[b % 2][1][:], scr3[1][:], 0.0)
                S2ready[b] = True
            if final[b_old] == idx:
                S2ready[b_old] = False

            # -- prefetch for the block entering next step: z1 and the
            #    S' partial sums over already-final inputs (c <= bb-2) --
            pre = [bb for bb in range(1, NB) if entry[bb] == idx + 1]
            for bb in pre:
                for j in range(NJ):
                    pg = xtt[j % 2]
                    nc.tensor.transpose(pg, xB[:, j, :], eye[:])
                    nc.vector.tensor_scalar_add(xT4[:, j, :], pg, 0.0)
                scr = None
                scr = pzz[bb % 2]
                mm(scr[0][:], w0[:, bb * P:(bb + 1) * P],
                   xT4[:, 0:NJ // 2, :], True, True)
                mm(scr[1][:], w0[:, bb * P:(bb + 1) * P],
                   xT4[:, NJ // 2:NJ, :], True, True)
                nc.scalar.copy(z1nL[:], scr[0][:])
                nc.vector.tensor_scalar_add(z1nR[:], scr[1][:], 0.0)
                cc2 = [c for c in range(NB) if nzh0[bb, c] and c <= bb - 2]
                cc3 = [c for c in range(NB) if nzh1[bb, c] and c <= bb - 2]
                if cc2:
                    scr2 = pzz[(bb + 1) % 2]
                    for side in range(2):
                        for k, c in enumerate(cc2):
                            mm(scr2[side][:], wh0[:, c, bb * P:(bb + 1) * P],
                               a1[side][c][:], k == 0, k == len(cc2) - 1)
                    nc.scalar.copy(S2p[bb % 2][0][:], scr2[0][:])
                    nc.vector.tensor_scalar_add(S2p[bb % 2][1][:],
                                                scr2[1][:], 0.0)
                if cc3:
                    for side in range(2):
                        for k, c in enumerate(cc3):
                            mm(scr[side][:], wh1[:, c, bb * P:(bb + 1) * P],
                               a2[side][c][:], k == 0, k == len(cc3) - 1)
                    nc.scalar.copy(S3p[bb % 2][0][:], scr[0][:])
                    nc.vector.tensor_scalar_add(S3p[bb % 2][1][:],
                                                scr[1][:], 0.0)

        # ---- output ----
        nc.scalar.copy(xBf[:], xB[:])
        nc.sync.dma_start(d_out, xBf[:])

    nc.compile()
    return nc


_CACHE = {}


def _get_program(prep):
    key = (DTYPE, STOP, hashlib.md5(prep["bout"].tobytes()).hexdigest())
    if key not in _CACHE:
        _CACHE[key] = _build(prep)
    return _CACHE[key]


def _run(inputs, trace=False):
    z = np.asarray(inputs["z"], dtype=np.float32)
    prep = _host_prep(np.asarray(inputs["W0"], np.float32),
                      np.asarray(inputs["b0"], np.float32),
                      np.asarray(inputs["Wh"], np.float32),
                      np.asarray(inputs["bh"], np.float32),
                      np.asarray(inputs["Wout"], np.float32),
                      np.asarray(inputs["bout"], np.float32))
    nc = _get_program(prep)

    eye = np.eye(P, dtype=NPDT)
    bout32 = prep["bout"].astype(np.float32)
    bml1 = np.empty(2 * D, dtype=np.float32)
    bml1[0::2] = bout32[:D]
    bml1[1::2] = bout32[D:]
    bml = np.ascontiguousarray(
        np.broadcast_to(np.tile(bml1, NJ), (P, NJ * 2 * D))).astype(np.float32)
    in_maps = []
    for c in range(NCORES):
        zs = z[c * BC:(c + 1) * BC, :]                     # [512, 32]
        # batch-major: [p, j, i] = z[j*128+p, i]
        zb = np.ascontiguousarray(
            zs.reshape(NJ, P, D).transpose(1, 0, 2).reshape(P, NJ * D)
        ).astype(NPDT)
        in_maps.append({
            "w0t": prep["W0T"], "wh0t": prep["Wh0T"], "wh1t": prep["Wh1T"],
            "wot": prep["WoIT"], "b0l": prep["b0L"], "bh0l": prep["bh0L"],
            "bh1l": prep["bh1L"], "zb": zb, "eye": eye,
            "w0r": prep["W0T"].reshape(1, D, H), "bml": bml,
        })

    res = run_bass_kernel_spmd(nc, in_maps, core_ids=list(range(NCORES)),
                               trace=trace)
    out = np.empty((B, D), dtype=np.float32)
    for c in range(NCORES):
        buf = res.results[c]["out"]                        # [128, 128]
        out[c * BC:(c + 1) * BC, :] = (
            buf.reshape(P, NJ, D).transpose(1, 0, 2).reshape(BC, D))
    return out, res


def kernel(**inputs):
    out, _ = _run(inputs, trace=False)
    return out
